# revision 15
# baseline (speedup 1.0000x reference)
"""Causal self-attention (B=4, T=2048, C=1024, 16 heads) on 8 trn2 NeuronCores.

Sharding: core i handles batch b=i//2 and head-half hh=i%2 (8 of 16 heads).
Each core computes its 8 heads' attention output projected through its slice
of W_proj rows (a partial sum of y); host adds the two head-half partials per
batch and transposes back.

Layout strategy (matmul operands in bf16, fp32 PSUM accumulation):
  - host pre-transposes x[b] -> xT [C, T]; xT and all weights live in SBUF
    for the whole kernel, loaded by a handful of large DMAs
  - qk^T = W_qk.T @ x (via lhsT=W_qk chunks, rhs=xT chunks): [qk_cols, T]
  - V natural [T, vcols] (via lhsT=xT chunk, rhs=W_v), with a ones column
    per head so the PV matmul also produces the softmax denominator
  - S^T[tk, tq] = K_h @ Q_h^T via lhsT=K^T cols, rhs=Q^T (two heads packed
    into the 128-row PE array with tile_position row groups); both heads of
    a pair share one [128, 1024] PSUM tile so exp runs as a single ACT op
  - causal mask added in PSUM by an identity-weight matmul of a -1e30 mask
  - P^T = exp(S^T/8) on ScalarE (masked entries underflow to exactly 0)
  - O^T[65, tq] accumulates lhsT=V_ext[tk,65], rhs=P^T; row 64 = sum(exp);
    O copies to SBUF immediately (frees the PSUM bank), then reciprocal of
    row 64 on DVE, gpsimd partition_broadcast, multiply on DVE
  - next block's QKV matmul groups are interleaved into the attention
    instruction stream so the in-order PE queue has filler work while
    ScalarE computes exp
  - y^T = W_proj.T @ attn_out^T accumulated over head pairs; bias per
    partition; one output DMA per block
"""

import sys

sys.path.insert(0, "/opt/trn_rl_repo")

import numpy as np
import ml_dtypes

BF16 = ml_dtypes.bfloat16

B, T, C = 4, 2048, 1024
NHEAD_GLOBAL = 16
D = 64
H = 8                    # local heads per core
HP = H // 2              # head pairs
NB = 4                   # tq blocks
BLK = T // NB            # 512
CCH = C // 128           # 8 contraction chunks
TCH = T // 128           # 16 tk chunks
NEG = -1.0e30

MASK_W = [128, 256, 384, 512]     # mask matmul widths per diagonal pos d
MASK_OFF = [0, 128, 384, 768]     # col offset of mask d in the packed input
SCORE_C0 = [0, 128, 256, 384]     # scores/pv matmul col start per diagonal pos d

_CACHE = {}


def _build_nc(variant=""):
    import concourse.bass as bass  # noqa: F401
    import concourse.mybir as mybir
    import concourse.tile as tile
    from concourse import bacc

    f32 = mybir.dt.float32
    bf = mybir.dt.bfloat16

    nc = bacc.Bacc("TRN2", target_bir_lowering=False, debug=False)

    xT = nc.dram_tensor("xT", [C, T], bf, kind="ExternalInput").ap()
    wqk = nc.dram_tensor("wqk", [C, 1024], bf, kind="ExternalInput").ap()
    wv = nc.dram_tensor("wv", [C, 512], bf, kind="ExternalInput").ap()
    wpr = nc.dram_tensor("wproj", [512, C], bf, kind="ExternalInput").ap()
    mpk = nc.dram_tensor("maskpack", [128, 1408], bf, kind="ExternalInput").ap()
    bias = nc.dram_tensor("biases", [128, 16], f32, kind="ExternalInput").ap()
    yT = nc.dram_tensor("yT", [C, T], f32, kind="ExternalOutput").ap()

    Exp = mybir.ActivationFunctionType.Exp
    Mult = mybir.AluOpType.mult

    with tile.TileContext(nc) as tc:
        with (
            tc.tile_pool(name="const", bufs=1) as cpool,
            tc.tile_pool(name="kv", bufs=1) as kvpool,
            tc.tile_pool(name="qt", bufs=2) as qtpool,
            tc.tile_pool(name="pt", bufs=6) as ptpool,
            tc.tile_pool(name="ot", bufs=2) as otpool,
            tc.tile_pool(name="osb", bufs=4) as osbpool,
            tc.tile_pool(name="ysb", bufs=2) as ypool,
            tc.tile_pool(name="rcp", bufs=3) as rpool,
            tc.tile_pool(name="pa_ps", bufs=2, space="PSUM") as papool,
            tc.tile_pool(name="st_ps", bufs=2, space="PSUM") as stpool,
            tc.tile_pool(name="o_ps", bufs=2, space="PSUM") as opool,
        ):
            # ---- resident inputs, loaded via a few large DMAs ----
            xt_all = cpool.tile([128, CCH, T], bf, tag="xt", name="xt_all")
            xr = xT.rearrange("(c p) t -> p c t", p=128)
            nc.sync.dma_start(xt_all[:, :4, :], xr[:, :4, :])
            nc.sync.dma_start(xt_all[:, 4:, :], xr[:, 4:, :])

            wqk_all = cpool.tile([128, CCH, 1024], bf, tag="wqk", name="wqk_all")
            wr = wqk.rearrange("(c p) n -> p c n", p=128)
            nc.sync.dma_start(wqk_all[:, :4, :], wr[:, :4, :])
            nc.sync.dma_start(wqk_all[:, 4:, :], wr[:, 4:, :])

            wv_all = cpool.tile([128, CCH, 512], bf, tag="wv", name="wv_all")
            nc.sync.dma_start(wv_all[:], wv.rearrange("(c p) n -> p c n", p=128))

            wpr_all = cpool.tile([128, 4, 1024], bf, tag="wpr", name="wpr_all")
            nc.sync.dma_start(wpr_all[:], wpr.rearrange("(c p) n -> p c n", p=128))

            mp_sb = cpool.tile([128, 1408], bf, tag="mpk", name="mp_sb")
            nc.sync.dma_start(mp_sb[:], mpk[:, :])
            mask_sb = [
                mp_sb[:, MASK_OFF[d] : MASK_OFF[d] + MASK_W[d]] for d in range(4)
            ]
            ident_sb = mp_sb[:, 1280:1408]

            bias_sb = cpool.tile([128, 16], f32, tag="bias", name="bias_sb")
            nc.sync.dma_start(bias_sb[:], bias[:, :])
            bqk_sb = bias_sb[:, 0:8]
            bpr_sb = bias_sb[:, 8:16]

            ones_sb = cpool.tile([128, 1], f32, tag="ones", name="ones")
            nc.vector.memset(ones_sb[:], 1.0)

            # persistent K^T tiles per (head-pair, block) and V tiles per tk chunk
            kT = [[None] * NB for _ in range(HP)]
            vt = [None] * TCH
            qT_store = [[None] * 4 for _ in range(NB)]

            def qk_group(j, t):
                ps = papool.tile([128, BLK], f32, tag="pa", name="pa")
                for c in range(CCH):
                    nc.tensor.matmul(
                        ps[:],
                        lhsT=wqk_all[:, c, t * 128 : (t + 1) * 128],
                        rhs=xt_all[:, c, j * BLK : (j + 1) * BLK],
                        start=(c == 0),
                        stop=(c == CCH - 1),
                    )
                if t < 4:
                    dst = qtpool.tile([128, BLK], bf, tag=f"qT{t}", name=f"qT{t}")
                    qT_store[j][t] = dst
                else:
                    dst = kvpool.tile(
                        [128, BLK], bf, tag=f"kT{t - 4}_{j}", name=f"kT{t - 4}_{j}"
                    )
                    kT[t - 4][j] = dst
                nc.vector.tensor_scalar_add(dst[:], ps[:], bqk_sb[:, t : t + 1])

            def v_group(j, tcl):
                tci = 4 * j + tcl
                ps = papool.tile([128, BLK], f32, tag="pa", name="pa")
                for c in range(CCH):
                    nc.tensor.matmul(
                        ps[:],
                        lhsT=xt_all[:, c, tci * 128 : (tci + 1) * 128],
                        rhs=wv_all[:, c, :],
                        start=(c == 0),
                        stop=(c == CCH - 1),
                    )
                v_ = kvpool.tile([128, H, 65], bf, tag=f"v{tci}", name=f"v{tci}")
                vt[tci] = v_
                nc.vector.tensor_copy(
                    v_[:, :, 64], ones_sb[:, 0:1].to_broadcast([128, H])
                )
                nc.vector.tensor_copy(
                    v_[:, :, :64], ps[:].rearrange("p (h d) -> p h d", d=64)
                )

            def qkv_thunks(j):
                return [lambda t=t: qk_group(j, t) for t in range(8)] + [
                    lambda tcl=tcl: v_group(j, tcl) for tcl in range(4)
                ]

            # prologue: block 0 QKV
            for th in qkv_thunks(0):
                th()

            for j in range(NB):
                nchunks = 4 * j + 4
                pending = qkv_thunks(j + 1) if j + 1 < NB else []
                emitted = 0
                total_iters = HP * nchunks
                it = 0

                oT = [
                    otpool.tile([128, BLK], bf, tag=f"oT{hp}", name=f"oT{hp}")
                    for hp in range(HP)
                ]
                qT = qT_store[j]
                for hp in range(HP):
                    o_ps = [
                        opool.tile([65, BLK], f32, tag="o", name="o_ps")
                        for _ in range(2)
                    ]
                    prev = None
                    for tci in range(nchunks):
                        d = tci - 4 * j
                        c0 = 0 if d < 0 else SCORE_C0[d]
                        st = stpool.tile([128, 2 * BLK], f32, tag="st", name="st")
                        for s in range(2):
                            sb = s * BLK
                            kslc = kT[hp][tci // 4][
                                64 * s : 64 * s + 64,
                                (tci % 4) * 128 : (tci % 4 + 1) * 128,
                            ]
                            if d >= 0:
                                nc.tensor.matmul(
                                    st[:, sb : sb + MASK_W[d]],
                                    lhsT=ident_sb,
                                    rhs=mask_sb[d],
                                    start=True,
                                    stop=False,
                                )
                            nc.tensor.matmul(
                                st[:, sb + c0 : sb + BLK],
                                lhsT=kslc,
                                rhs=qT[hp][64 * s : 64 * s + 64, c0:],
                                start=(d < 0),
                                stop=True,
                                tile_position=(64 * s, 0),
                            )
                        pt = ptpool.tile([128, 2 * BLK], bf, tag="pT", name="pT")
                        nc.scalar.activation(pt[:], st[:], Exp, scale=0.125)
                        if prev is not None:
                            pt_p, tcp, c0p = prev
                            for s in range(2):
                                nc.tensor.matmul(
                                    o_ps[s][:, c0p:],
                                    lhsT=vt[tcp][:, 2 * hp + s, :],
                                    rhs=pt_p[:, s * BLK + c0p : (s + 1) * BLK],
                                    start=(tcp == 0),
                                    stop=(tcp == nchunks - 1),
                                )
                        prev = (pt, tci, c0)
                        # interleave next block's QKV groups into the PE stream
                        it += 1
                        want = (it * len(pending)) // total_iters if pending else 0
                        while emitted < want:
                            pending[emitted]()
                            emitted += 1
                    pt_p, tcp, c0p = prev
                    for s in range(2):
                        nc.tensor.matmul(
                            o_ps[s][:, c0p:],
                            lhsT=vt[tcp][:, 2 * hp + s, :],
                            rhs=pt_p[:, s * BLK + c0p : (s + 1) * BLK],
                            start=(tcp == 0),
                            stop=(tcp == nchunks - 1),
                        )
                    for s in range(2):
                        o_sb = osbpool.tile([65, BLK], f32, tag="osb", name="o_sb")
                        nc.vector.tensor_copy(o_sb[:], o_ps[s][:])
                        rc = rpool.tile([1, BLK], f32, tag="rcp", name="rcp")
                        nc.vector.reciprocal(rc[:], o_sb[64:65, :])
                        bc = rpool.tile([64, BLK], f32, tag="bc", name="bc")
                        nc.gpsimd.partition_broadcast(bc[:], rc[:])
                        nc.vector.tensor_tensor(
                            oT[hp][64 * s : 64 * s + 64, :],
                            o_sb[:64, :],
                            bc[:],
                            Mult,
                        )
                while emitted < len(pending):
                    pending[emitted]()
                    emitted += 1

                # ---------------- output projection for block j ----------------
                ysb = ypool.tile([128, 8, BLK], f32, tag="y", name="ysb")
                for t in range(8):
                    ps = papool.tile([128, BLK], f32, tag="pa", name="pa")
                    for cp in range(4):
                        nc.tensor.matmul(
                            ps[:],
                            lhsT=wpr_all[:, cp, t * 128 : (t + 1) * 128],
                            rhs=oT[cp][:],
                            start=(cp == 0),
                            stop=(cp == 3),
                        )
                    nc.vector.tensor_scalar_add(
                        ysb[:, t, :], ps[:], bpr_sb[:, t : t + 1]
                    )
                nc.sync.dma_start(
                    yT.rearrange("(t p) q -> p t q", p=128)[
                        :, :, j * BLK : (j + 1) * BLK
                    ],
                    ysb[:],
                )

    nc.compile()
    return nc


def _host_inputs(x, W_attn, b_attn, W_proj, b_proj):
    """Build the 8 per-core input maps."""
    x = np.asarray(x, dtype=np.float32)
    W_attn = np.asarray(W_attn, dtype=np.float32)
    b_attn = np.asarray(b_attn, dtype=np.float32)
    W_proj = np.asarray(W_proj, dtype=np.float32)
    b_proj = np.asarray(b_proj, dtype=np.float32)

    p = np.arange(128)[:, None]
    maskpack = np.zeros((128, 1408), dtype=np.float32)
    for d in range(4):
        f = np.arange(MASK_W[d])[None, :]
        maskpack[:, MASK_OFF[d] : MASK_OFF[d] + MASK_W[d]] = np.where(
            f >= 128 * d + p, 0.0, NEG
        )
    maskpack[:, 1280:1408] = np.eye(128, dtype=np.float32)
    maskpack = maskpack.astype(BF16)

    in_maps = []
    for core in range(8):
        b = core // 2
        hh = core % 2
        cs = hh * 512
        wq = W_attn[:, cs : cs + 512]
        wk = W_attn[:, C + cs : C + cs + 512]
        wv_ = W_attn[:, 2 * C + cs : 2 * C + cs + 512]
        bq = b_attn[cs : cs + 512]
        bk = b_attn[C + cs : C + cs + 512]
        bv = b_attn[2 * C + cs : 2 * C + cs + 512]
        wpr_ = W_proj[cs : cs + 512, :]
        bpr_eff = bv @ wpr_ + (b_proj if hh == 0 else 0.0)
        biases = np.concatenate(
            [
                np.concatenate([bq, bk]).reshape(8, 128).T,
                bpr_eff.astype(np.float32).reshape(8, 128).T,
            ],
            axis=1,
        )
        in_maps.append(
            {
                "xT": np.ascontiguousarray(x[b].T).astype(BF16),
                "wqk": np.ascontiguousarray(
                    np.concatenate([wq, wk], axis=1)
                ).astype(BF16),
                "wv": np.ascontiguousarray(wv_).astype(BF16),
                "wproj": np.ascontiguousarray(wpr_).astype(BF16),
                "maskpack": maskpack,
                "biases": np.ascontiguousarray(biases.astype(np.float32)),
            }
        )
    return in_maps


def run(x, W_attn, b_attn, W_proj, b_proj, trace=False):
    from concourse.bass_utils import run_bass_kernel_spmd

    if "nc" not in _CACHE:
        _CACHE["nc"] = _build_nc()
    nc = _CACHE["nc"]
    in_maps = _host_inputs(x, W_attn, b_attn, W_proj, b_proj)
    res = run_bass_kernel_spmd(nc, in_maps, core_ids=list(range(8)), trace=False)
    y = np.empty((B, T, C), dtype=np.float32)
    for b in range(B):
        y[b] = (res.results[2 * b]["yT"] + res.results[2 * b + 1]["yT"]).T
    return y, res


def kernel(x, W_attn, b_attn, W_proj, b_proj):
    y, _ = run(x, W_attn, b_attn, W_proj, b_proj, trace=False)
    return y


def make_timed_runner(in_maps=None, nc=None):
    """Build a non-donating jitted SPMD callable with device-resident inputs.

    Returns fn(n) -> wall seconds to execute the kernel n times back-to-back
    (async dispatch, single block at the end). Differential timing
    (wall(n) - wall(1)) / (n - 1) estimates per-execution device time plus
    a per-call dispatch overhead (~400us on this axon setup; calibrate with
    a trivial kernel and subtract).
    """
    import jax
    import numpy as _np
    import concourse.mybir as mybir
    from concourse import bass2jax
    from jax.experimental.shard_map import shard_map
    from jax.sharding import Mesh, PartitionSpec, NamedSharding

    if nc is None:
        if "nc" not in _CACHE:
            _CACHE["nc"] = _build_nc()
        nc = _CACHE["nc"]

    bass2jax.install_neuronx_cc_hook()
    n_cores = 8

    partition_name = nc.partition_id_tensor.name if nc.partition_id_tensor else None
    in_names, out_names, out_avals, zero_outs = [], [], [], []
    for alloc in nc.m.functions[0].allocations:
        if not isinstance(alloc, mybir.MemoryLocationSet):
            continue
        name = alloc.memorylocations[0].name
        if alloc.kind == "ExternalInput":
            if name != partition_name:
                in_names.append(name)
        elif alloc.kind == "ExternalOutput":
            out_names.append(name)
            shape = tuple(alloc.tensor_shape)
            dtype = mybir.dt.np(alloc.dtype)
            out_avals.append(jax.core.ShapedArray(shape, dtype))
            zero_outs.append(_np.zeros(shape, dtype))
    n_params = len(in_names)
    all_names = in_names + out_names
    if partition_name is not None:
        all_names = all_names + [partition_name]

    def _body(*args):
        operands = list(args)
        if partition_name is not None:
            operands.append(bass2jax.partition_id_tensor())
        outs = bass2jax._bass_exec_p.bind(
            *operands,
            out_avals=tuple(out_avals),
            in_names=tuple(all_names),
            out_names=tuple(out_names),
            lowering_input_output_aliases=(),
            sim_require_finite=True,
            sim_require_nnan=True,
            nc=nc,
        )
        return tuple(outs)

    devices = jax.devices()[:n_cores]
    mesh = Mesh(_np.asarray(devices), ("core",))
    spec = PartitionSpec("core")
    sharded = jax.jit(
        shard_map(
            _body,
            mesh=mesh,
            in_specs=(spec,) * (n_params + len(out_names)),
            out_specs=(spec,) * len(out_names),
            check_rep=False,
        ),
        keep_unused=True,
    )
    sh = NamedSharding(mesh, spec)
    dev_args = [
        jax.device_put(
            _np.concatenate([_np.asarray(in_maps[c][nm]) for c in range(n_cores)], 0),
            sh,
        )
        for nm in in_names
    ] + [
        jax.device_put(
            _np.zeros((n_cores * z.shape[0], *z.shape[1:]), z.dtype), sh
        )
        for z in zero_outs
    ]

    import time as _time

    def timed(n):
        out = None
        t0 = _time.perf_counter()
        for _ in range(n):
            out = sharded(*dev_args)
        jax.block_until_ready(out)
        return _time.perf_counter() - t0

    return timed


# revision 17
# speedup vs baseline: 1.1951x; 1.1951x over previous
"""Causal self-attention (B=4, T=2048, C=1024, 16 heads) on 8 trn2 NeuronCores.

Sharding: core i handles batch b=i//2 and head-half hh=i%2 (8 of 16 heads).
Each core computes its 8 heads' attention output projected through its slice
of W_proj rows (a partial sum of y); host adds the two head-half partials per
batch and transposes back.

Layout strategy (matmul operands in bf16, fp32 PSUM accumulation):
  - host pre-transposes x[b] -> xT [C, T]; xT and all weights live in SBUF
    for the whole kernel, loaded by a handful of large DMAs
  - qk^T = W_qk.T @ x (via lhsT=W_qk chunks, rhs=xT chunks): [qk_cols, T]
  - V natural [T, vcols] (via lhsT=xT chunk, rhs=W_v), with a ones column
    per head so the PV matmul also produces the softmax denominator
  - S^T[tk, tq] = K_h @ Q_h^T via lhsT=K^T cols, rhs=Q^T (two heads packed
    into the 128-row PE array with tile_position row groups); both heads of
    a pair share one [128, 1024] PSUM tile so exp runs as a single ACT op
  - causal mask added in PSUM by an identity-weight matmul of a -1e30 mask
  - P^T = exp(S^T/8) on ScalarE (masked entries underflow to exactly 0)
  - O^T[65, tq] accumulates lhsT=V_ext[tk,65], rhs=P^T; row 64 = sum(exp);
    O copies to SBUF immediately (frees the PSUM bank), then reciprocal of
    row 64 on DVE, gpsimd partition_broadcast, multiply on DVE
  - next block's QKV matmul groups are interleaved into the attention
    instruction stream so the in-order PE queue has filler work while
    ScalarE computes exp
  - y^T = W_proj.T @ attn_out^T accumulated over head pairs; bias per
    partition; one output DMA per block
"""

import sys

sys.path.insert(0, "/opt/trn_rl_repo")

import numpy as np
import ml_dtypes

BF16 = ml_dtypes.bfloat16

B, T, C = 4, 2048, 1024
NHEAD_GLOBAL = 16
D = 64
H = 8                    # local heads per core
HP = H // 2              # head pairs
NB = 4                   # tq blocks
BLK = T // NB            # 512
CCH = C // 128           # 8 contraction chunks
TCH = T // 128           # 16 tk chunks
NEG = -1.0e30

MASK_W = [128, 256, 384, 512]     # mask matmul widths per diagonal pos d
MASK_OFF = [0, 128, 384, 768]     # col offset of mask d in the packed input
SCORE_C0 = [0, 128, 256, 384]     # scores/pv matmul col start per diagonal pos d

_CACHE = {}


def _build_nc(variant=""):
    import concourse.bass as bass  # noqa: F401
    import concourse.mybir as mybir
    import concourse.tile as tile
    from concourse import bacc

    f32 = mybir.dt.float32
    bf = mybir.dt.bfloat16

    nc = bacc.Bacc("TRN2", target_bir_lowering=False, debug=False)

    xT = nc.dram_tensor("xT", [128, NB * CCH * BLK], bf, kind="ExternalInput").ap()
    wqk = nc.dram_tensor("wqk", [128, CCH * 1024], bf, kind="ExternalInput").ap()
    wv = nc.dram_tensor("wv", [128, CCH * 512], bf, kind="ExternalInput").ap()
    wpr = nc.dram_tensor("wproj", [128, 4 * 1024], bf, kind="ExternalInput").ap()
    mpk = nc.dram_tensor("maskpack", [128, 1408], bf, kind="ExternalInput").ap()
    bias = nc.dram_tensor("biases", [128, 16], f32, kind="ExternalInput").ap()
    yT = nc.dram_tensor("yT", [128, NB * 8 * BLK], bf, kind="ExternalOutput").ap()

    Exp = mybir.ActivationFunctionType.Exp
    Mult = mybir.AluOpType.mult

    with tile.TileContext(nc) as tc:
        with (
            tc.tile_pool(name="const", bufs=1) as cpool,
            tc.tile_pool(name="kv", bufs=1) as kvpool,
            tc.tile_pool(name="qt", bufs=2) as qtpool,
            tc.tile_pool(name="pt", bufs=6) as ptpool,
            tc.tile_pool(name="ot", bufs=2) as otpool,
            tc.tile_pool(name="osb", bufs=4) as osbpool,
            tc.tile_pool(name="ysb", bufs=2) as ypool,
            tc.tile_pool(name="rcp", bufs=3) as rpool,
            tc.tile_pool(name="pa_ps", bufs=2, space="PSUM") as papool,
            tc.tile_pool(name="st_ps", bufs=2, space="PSUM") as stpool,
            tc.tile_pool(name="o_ps", bufs=2, space="PSUM") as opool,
        ):
            # ---- resident inputs: flat 2D DMAs in need-order ----
            # SP ring: block-0 x slab, then wqk halves, then remaining x slabs
            xt_all = cpool.tile([128, NB * CCH * BLK], bf, tag="xt", name="xt_all")
            wqk_all = cpool.tile([128, CCH * 1024], bf, tag="wqk", name="wqk_all")
            XS = CCH * BLK
            nc.sync.dma_start(xt_all[:, :XS], xT[:, :XS])
            nc.sync.dma_start(wqk_all[:, : 4 * 1024], wqk[:, : 4 * 1024])
            nc.sync.dma_start(wqk_all[:, 4 * 1024 :], wqk[:, 4 * 1024 :])
            for jj in range(1, NB):
                nc.sync.dma_start(
                    xt_all[:, jj * XS : (jj + 1) * XS], xT[:, jj * XS : (jj + 1) * XS]
                )

            # SWDGE ring: everything else, in need-order
            wv_all = cpool.tile([128, CCH * 512], bf, tag="wv", name="wv_all")
            nc.gpsimd.dma_start(wv_all[:], wv[:, :])
            mp_sb = cpool.tile([128, 1408], bf, tag="mpk", name="mp_sb")
            nc.gpsimd.dma_start(mp_sb[:], mpk[:, :])
            mask_sb = [
                mp_sb[:, MASK_OFF[d] : MASK_OFF[d] + MASK_W[d]] for d in range(4)
            ]
            ident_sb = mp_sb[:, 1280:1408]
            bias_sb = cpool.tile([128, 16], f32, tag="bias", name="bias_sb")
            nc.gpsimd.dma_start(bias_sb[:], bias[:, :])
            wpr_all = cpool.tile([128, 4 * 1024], bf, tag="wpr", name="wpr_all")
            nc.gpsimd.dma_start(wpr_all[:], wpr[:, :])
            bqk_sb = bias_sb[:, 0:8]
            bpr_sb = bias_sb[:, 8:16]

            def xts(jj, c):
                off = (jj * CCH + c) * BLK
                return xt_all[:, off : off + BLK]

            ones_sb = cpool.tile([128, 1], f32, tag="ones", name="ones")
            nc.vector.memset(ones_sb[:], 1.0)

            # persistent K^T tiles per (head-pair, block) and V tiles per tk chunk
            kT = [[None] * NB for _ in range(HP)]
            vt = [None] * TCH
            qT_store = [[None] * 4 for _ in range(NB)]

            def qk_group(j, t):
                ps = papool.tile([128, BLK], f32, tag="pa", name="pa")
                for c in range(CCH):
                    nc.tensor.matmul(
                        ps[:],
                        lhsT=wqk_all[:, c * 1024 + t * 128 : c * 1024 + (t + 1) * 128],
                        rhs=xts(j, c),
                        start=(c == 0),
                        stop=(c == CCH - 1),
                    )
                if t < 4:
                    dst = qtpool.tile([128, BLK], bf, tag=f"qT{t}", name=f"qT{t}")
                    qT_store[j][t] = dst
                else:
                    dst = kvpool.tile(
                        [128, BLK], bf, tag=f"kT{t - 4}_{j}", name=f"kT{t - 4}_{j}"
                    )
                    kT[t - 4][j] = dst
                nc.vector.tensor_scalar_add(dst[:], ps[:], bqk_sb[:, t : t + 1])

            def v_group(j, tcl):
                tci = 4 * j + tcl
                ps = papool.tile([128, BLK], f32, tag="pa", name="pa")
                for c in range(CCH):
                    nc.tensor.matmul(
                        ps[:],
                        lhsT=xts(j, c)[:, tcl * 128 : (tcl + 1) * 128],
                        rhs=wv_all[:, c * 512 : (c + 1) * 512],
                        start=(c == 0),
                        stop=(c == CCH - 1),
                    )
                v_ = kvpool.tile([128, H, 65], bf, tag=f"v{tci}", name=f"v{tci}")
                vt[tci] = v_
                nc.vector.tensor_copy(
                    v_[:, :, 64], ones_sb[:, 0:1].to_broadcast([128, H])
                )
                nc.vector.tensor_copy(
                    v_[:, :, :64], ps[:].rearrange("p (h d) -> p h d", d=64)
                )

            def qkv_thunks(j):
                return [lambda t=t: qk_group(j, t) for t in range(8)] + [
                    lambda tcl=tcl: v_group(j, tcl) for tcl in range(4)
                ]

            # prologue: block 0 QKV
            for th in qkv_thunks(0):
                th()

            for j in range(NB):
                nchunks = 4 * j + 4
                pending = qkv_thunks(j + 1) if j + 1 < NB else []
                emitted = 0
                total_iters = HP * nchunks
                it = 0

                oT = [
                    otpool.tile([128, BLK], bf, tag=f"oT{hp}", name=f"oT{hp}")
                    for hp in range(HP)
                ]
                qT = qT_store[j]
                for hp in range(HP):
                    o_ps = [
                        opool.tile([65, BLK], f32, tag="o", name="o_ps")
                        for _ in range(2)
                    ]
                    prev = None
                    for tci in range(nchunks):
                        d = tci - 4 * j
                        c0 = 0 if d < 0 else SCORE_C0[d]
                        st = stpool.tile([128, 2 * BLK], f32, tag="st", name="st")
                        for s in range(2):
                            sb = s * BLK
                            kslc = kT[hp][tci // 4][
                                64 * s : 64 * s + 64,
                                (tci % 4) * 128 : (tci % 4 + 1) * 128,
                            ]
                            if d >= 0:
                                nc.tensor.matmul(
                                    st[:, sb : sb + MASK_W[d]],
                                    lhsT=ident_sb,
                                    rhs=mask_sb[d],
                                    start=True,
                                    stop=False,
                                )
                            nc.tensor.matmul(
                                st[:, sb + c0 : sb + BLK],
                                lhsT=kslc,
                                rhs=qT[hp][64 * s : 64 * s + 64, c0:],
                                start=(d < 0),
                                stop=True,
                                tile_position=(64 * s, 0),
                            )
                        pt = ptpool.tile([128, 2 * BLK], bf, tag="pT", name="pT")
                        nc.scalar.activation(pt[:], st[:], Exp, scale=0.125)
                        if prev is not None:
                            pt_p, tcp, c0p = prev
                            for s in range(2):
                                nc.tensor.matmul(
                                    o_ps[s][:, c0p:],
                                    lhsT=vt[tcp][:, 2 * hp + s, :],
                                    rhs=pt_p[:, s * BLK + c0p : (s + 1) * BLK],
                                    start=(tcp == 0),
                                    stop=(tcp == nchunks - 1),
                                )
                        prev = (pt, tci, c0)
                        # interleave next block's QKV groups into the PE stream
                        it += 1
                        want = (it * len(pending)) // total_iters if pending else 0
                        while emitted < want:
                            pending[emitted]()
                            emitted += 1
                    pt_p, tcp, c0p = prev
                    for s in range(2):
                        nc.tensor.matmul(
                            o_ps[s][:, c0p:],
                            lhsT=vt[tcp][:, 2 * hp + s, :],
                            rhs=pt_p[:, s * BLK + c0p : (s + 1) * BLK],
                            start=(tcp == 0),
                            stop=(tcp == nchunks - 1),
                        )
                    for s in range(2):
                        o_sb = osbpool.tile([65, BLK], f32, tag="osb", name="o_sb")
                        nc.vector.tensor_copy(o_sb[:], o_ps[s][:])
                        rc = rpool.tile([1, BLK], f32, tag="rcp", name="rcp")
                        nc.vector.reciprocal(rc[:], o_sb[64:65, :])
                        bc = rpool.tile([64, BLK], f32, tag="bc", name="bc")
                        nc.gpsimd.partition_broadcast(bc[:], rc[:])
                        nc.vector.tensor_tensor(
                            oT[hp][64 * s : 64 * s + 64, :],
                            o_sb[:64, :],
                            bc[:],
                            Mult,
                        )
                while emitted < len(pending):
                    pending[emitted]()
                    emitted += 1

                # ---------------- output projection for block j ----------------
                ysb = ypool.tile([128, 8 * BLK], bf, tag="y", name="ysb")
                for t in range(8):
                    ps = papool.tile([128, BLK], f32, tag="pa", name="pa")
                    for cp in range(4):
                        nc.tensor.matmul(
                            ps[:],
                            lhsT=wpr_all[
                                :, cp * 1024 + t * 128 : cp * 1024 + (t + 1) * 128
                            ],
                            rhs=oT[cp][:],
                            start=(cp == 0),
                            stop=(cp == 3),
                        )
                    nc.vector.tensor_scalar_add(
                        ysb[:, t * BLK : (t + 1) * BLK], ps[:], bpr_sb[:, t : t + 1]
                    )
                nc.gpsimd.dma_start(
                    yT[:, j * 8 * BLK : (j + 1) * 8 * BLK], ysb[:]
                )

    nc.compile()
    return nc


def _host_inputs(x, W_attn, b_attn, W_proj, b_proj):
    """Build the 8 per-core input maps."""
    x = np.asarray(x, dtype=np.float32)
    W_attn = np.asarray(W_attn, dtype=np.float32)
    b_attn = np.asarray(b_attn, dtype=np.float32)
    W_proj = np.asarray(W_proj, dtype=np.float32)
    b_proj = np.asarray(b_proj, dtype=np.float32)

    p = np.arange(128)[:, None]
    maskpack = np.zeros((128, 1408), dtype=np.float32)
    for d in range(4):
        f = np.arange(MASK_W[d])[None, :]
        maskpack[:, MASK_OFF[d] : MASK_OFF[d] + MASK_W[d]] = np.where(
            f >= 128 * d + p, 0.0, NEG
        )
    maskpack[:, 1280:1408] = np.eye(128, dtype=np.float32)
    maskpack = maskpack.astype(BF16)

    in_maps = []
    for core in range(8):
        b = core // 2
        hh = core % 2
        cs = hh * 512
        wq = W_attn[:, cs : cs + 512]
        wk = W_attn[:, C + cs : C + cs + 512]
        wv_ = W_attn[:, 2 * C + cs : 2 * C + cs + 512]
        bq = b_attn[cs : cs + 512]
        bk = b_attn[C + cs : C + cs + 512]
        bv = b_attn[2 * C + cs : 2 * C + cs + 512]
        wpr_ = W_proj[cs : cs + 512, :]
        bpr_eff = bv @ wpr_ + (b_proj if hh == 0 else 0.0)
        biases = np.concatenate(
            [
                np.concatenate([bq, bk]).reshape(8, 128).T,
                bpr_eff.astype(np.float32).reshape(8, 128).T,
            ],
            axis=1,
        )
        # xT packed: [128, (j, c, f)] with xT_pack[p, j, c, f] = x[b][j*BLK+f, c*128+p]
        xt = x[b].T.reshape(CCH, 128, NB, BLK)          # [c, p, j, f]
        xt_pack = np.ascontiguousarray(xt.transpose(1, 2, 0, 3)).reshape(
            128, NB * CCH * BLK
        )
        # weights packed: [128, (c, n)] with w_pack[p, c, n] = w[c*128+p, n]
        wqk_n = np.concatenate([wq, wk], axis=1)        # [C, 1024]
        wqk_pack = np.ascontiguousarray(
            wqk_n.reshape(CCH, 128, 1024).transpose(1, 0, 2)
        ).reshape(128, CCH * 1024)
        wv_pack = np.ascontiguousarray(
            wv_.reshape(CCH, 128, 512).transpose(1, 0, 2)
        ).reshape(128, CCH * 512)
        wpr_pack = np.ascontiguousarray(
            wpr_.reshape(4, 128, 1024).transpose(1, 0, 2)
        ).reshape(128, 4 * 1024)
        in_maps.append(
            {
                "xT": xt_pack.astype(BF16),
                "wqk": wqk_pack.astype(BF16),
                "wv": wv_pack.astype(BF16),
                "wproj": wpr_pack.astype(BF16),
                "maskpack": maskpack,
                "biases": np.ascontiguousarray(biases.astype(np.float32)),
            }
        )
    return in_maps


def run(x, W_attn, b_attn, W_proj, b_proj, trace=False):
    from concourse.bass_utils import run_bass_kernel_spmd

    if "nc" not in _CACHE:
        _CACHE["nc"] = _build_nc()
    nc = _CACHE["nc"]
    in_maps = _host_inputs(x, W_attn, b_attn, W_proj, b_proj)
    res = run_bass_kernel_spmd(nc, in_maps, core_ids=list(range(8)), trace=False)
    y = np.empty((B, T, C), dtype=np.float32)
    for b in range(B):
        # yT packed per core: [128, (j, t, f)] = y^T[t*128+p, j*BLK+f]
        acc = None
        for hh in range(2):
            yt = res.results[2 * b + hh]["yT"].astype(np.float32)
            yt = yt.reshape(128, NB, 8, BLK).transpose(2, 0, 1, 3).reshape(C, T)
            acc = yt if acc is None else acc + yt
        y[b] = acc.T
    return y, res


def kernel(x, W_attn, b_attn, W_proj, b_proj):
    y, _ = run(x, W_attn, b_attn, W_proj, b_proj, trace=False)
    return y


def make_timed_runner(in_maps=None, nc=None):
    """Build a non-donating jitted SPMD callable with device-resident inputs.

    Returns fn(n) -> wall seconds to execute the kernel n times back-to-back
    (async dispatch, single block at the end). Differential timing
    (wall(n) - wall(1)) / (n - 1) estimates per-execution device time plus
    a per-call dispatch overhead (~400us on this axon setup; calibrate with
    a trivial kernel and subtract).
    """
    import jax
    import numpy as _np
    import concourse.mybir as mybir
    from concourse import bass2jax
    from jax.experimental.shard_map import shard_map
    from jax.sharding import Mesh, PartitionSpec, NamedSharding

    if nc is None:
        if "nc" not in _CACHE:
            _CACHE["nc"] = _build_nc()
        nc = _CACHE["nc"]

    bass2jax.install_neuronx_cc_hook()
    n_cores = 8

    partition_name = nc.partition_id_tensor.name if nc.partition_id_tensor else None
    in_names, out_names, out_avals, zero_outs = [], [], [], []
    for alloc in nc.m.functions[0].allocations:
        if not isinstance(alloc, mybir.MemoryLocationSet):
            continue
        name = alloc.memorylocations[0].name
        if alloc.kind == "ExternalInput":
            if name != partition_name:
                in_names.append(name)
        elif alloc.kind == "ExternalOutput":
            out_names.append(name)
            shape = tuple(alloc.tensor_shape)
            dtype = mybir.dt.np(alloc.dtype)
            out_avals.append(jax.core.ShapedArray(shape, dtype))
            zero_outs.append(_np.zeros(shape, dtype))
    n_params = len(in_names)
    all_names = in_names + out_names
    if partition_name is not None:
        all_names = all_names + [partition_name]

    def _body(*args):
        operands = list(args)
        if partition_name is not None:
            operands.append(bass2jax.partition_id_tensor())
        outs = bass2jax._bass_exec_p.bind(
            *operands,
            out_avals=tuple(out_avals),
            in_names=tuple(all_names),
            out_names=tuple(out_names),
            lowering_input_output_aliases=(),
            sim_require_finite=True,
            sim_require_nnan=True,
            nc=nc,
        )
        return tuple(outs)

    devices = jax.devices()[:n_cores]
    mesh = Mesh(_np.asarray(devices), ("core",))
    spec = PartitionSpec("core")
    sharded = jax.jit(
        shard_map(
            _body,
            mesh=mesh,
            in_specs=(spec,) * (n_params + len(out_names)),
            out_specs=(spec,) * len(out_names),
            check_rep=False,
        ),
        keep_unused=True,
    )
    sh = NamedSharding(mesh, spec)
    dev_args = [
        jax.device_put(
            _np.concatenate([_np.asarray(in_maps[c][nm]) for c in range(n_cores)], 0),
            sh,
        )
        for nm in in_names
    ] + [
        jax.device_put(
            _np.zeros((n_cores * z.shape[0], *z.shape[1:]), z.dtype), sh
        )
        for z in zero_outs
    ]

    import time as _time

    def timed(n):
        out = None
        t0 = _time.perf_counter()
        for _ in range(n):
            out = sharded(*dev_args)
        jax.block_until_ready(out)
        return _time.perf_counter() - t0

    return timed


# revision 18
# speedup vs baseline: 1.6509x; 1.3814x over previous
"""Causal self-attention (B=4, T=2048, C=1024, 16 heads) on 8 trn2 NeuronCores.

Sharding: core i handles batch b=i//2 and head-half hh=i%2 (8 of 16 heads).
Each core computes its 8 heads' attention output projected through its slice
of W_proj rows (a partial sum of y); host adds the two head-half partials per
batch and transposes back.

Layout strategy (matmul operands in bf16, fp32 PSUM accumulation):
  - host pre-transposes x[b] -> xT [C, T]; xT and all weights live in SBUF
    for the whole kernel, loaded by a handful of large DMAs
  - qk^T = W_qk.T @ x (via lhsT=W_qk chunks, rhs=xT chunks): [qk_cols, T]
  - V natural [T, vcols] (via lhsT=xT chunk, rhs=W_v), with a ones column
    per head so the PV matmul also produces the softmax denominator
  - S^T[tk, tq] = K_h @ Q_h^T via lhsT=K^T cols, rhs=Q^T (two heads packed
    into the 128-row PE array with tile_position row groups); both heads of
    a pair share one [128, 1024] PSUM tile so exp runs as a single ACT op
  - causal mask added in PSUM by an identity-weight matmul of a -1e30 mask
  - P^T = exp(S^T/8) on ScalarE (masked entries underflow to exactly 0)
  - O^T[65, tq] accumulates lhsT=V_ext[tk,65], rhs=P^T; row 64 = sum(exp);
    O copies to SBUF immediately (frees the PSUM bank), then reciprocal of
    row 64 on DVE, gpsimd partition_broadcast, multiply on DVE
  - next block's QKV matmul groups are interleaved into the attention
    instruction stream so the in-order PE queue has filler work while
    ScalarE computes exp
  - y^T = W_proj.T @ attn_out^T accumulated over head pairs; bias per
    partition; one output DMA per block
"""

import sys

sys.path.insert(0, "/opt/trn_rl_repo")

import numpy as np
import ml_dtypes

BF16 = ml_dtypes.bfloat16

B, T, C = 4, 2048, 1024
NHEAD_GLOBAL = 16
D = 64
H = 8                    # local heads per core
HP = H // 2              # head pairs
NB = 4                   # tq blocks
BLK = T // NB            # 512
CCH = C // 128           # 8 contraction chunks
TCH = T // 128           # 16 tk chunks
NEG = -1.0e30

MASK_W = [128, 256, 384, 512]     # mask matmul widths per diagonal pos d
MASK_OFF = [0, 128, 384, 768]     # col offset of mask d in the packed input
SCORE_C0 = [0, 128, 256, 384]     # scores/pv matmul col start per diagonal pos d

_CACHE = {}


def _build_nc(variant=""):
    import concourse.bass as bass  # noqa: F401
    import concourse.mybir as mybir
    import concourse.tile as tile
    from concourse import bacc

    f32 = mybir.dt.float32
    bf = mybir.dt.bfloat16

    nc = bacc.Bacc("TRN2", target_bir_lowering=False, debug=False)

    xT = nc.dram_tensor("xT", [128, NB * CCH * BLK], bf, kind="ExternalInput").ap()
    wqk = nc.dram_tensor("wqk", [128, CCH * 1024], bf, kind="ExternalInput").ap()
    wv = nc.dram_tensor("wv", [128, CCH * 512], bf, kind="ExternalInput").ap()
    wpr = nc.dram_tensor("wproj", [128, 4 * 1024], bf, kind="ExternalInput").ap()
    mpk = nc.dram_tensor("maskpack", [128, 1408], bf, kind="ExternalInput").ap()
    bias = nc.dram_tensor("biases", [128, 16], f32, kind="ExternalInput").ap()
    yT = nc.dram_tensor("yT", [128, NB * 8 * BLK], bf, kind="ExternalOutput").ap()

    Exp = mybir.ActivationFunctionType.Exp
    Mult = mybir.AluOpType.mult

    with tile.TileContext(nc) as tc:
        with (
            tc.tile_pool(name="const", bufs=1) as cpool,
            tc.tile_pool(name="kv", bufs=1) as kvpool,
            tc.tile_pool(name="qt", bufs=2) as qtpool,
            tc.tile_pool(name="pt", bufs=6) as ptpool,
            tc.tile_pool(name="ot", bufs=2) as otpool,
            tc.tile_pool(name="osb", bufs=4) as osbpool,
            tc.tile_pool(name="ysb", bufs=2) as ypool,
            tc.tile_pool(name="rcp", bufs=3) as rpool,
            tc.tile_pool(name="pa_ps", bufs=2, space="PSUM") as papool,
            tc.tile_pool(name="st_ps", bufs=2, space="PSUM") as stpool,
            tc.tile_pool(name="o_ps", bufs=2, space="PSUM") as opool,
        ):
            # ---- resident inputs: flat 2D DMAs in dependency order ----
            xt_all = cpool.tile([128, NB * CCH * BLK], bf, tag="xt", name="xt_all")
            wqk_all = cpool.tile([128, CCH * 1024], bf, tag="wqk", name="wqk_all")
            XS = CCH * BLK

            def xts(jj, c):
                off = (jj * CCH + c) * BLK
                return xt_all[:, off : off + BLK]

            # SP ring: interleave block-0 x chunks with wqk chunks so the
            # first qk matmuls start after ~0.5MB has landed, then stream
            # the remaining x blocks behind compute.
            for c in range(CCH):
                nc.sync.dma_start(xts(0, c), xT[:, c * BLK : (c + 1) * BLK])
                nc.sync.dma_start(
                    wqk_all[:, c * 1024 : (c + 1) * 1024],
                    wqk[:, c * 1024 : (c + 1) * 1024],
                )
            for jj in range(1, NB):
                for ch in range(2):
                    lo = jj * XS + ch * (XS // 2)
                    nc.sync.dma_start(
                        xt_all[:, lo : lo + XS // 2], xT[:, lo : lo + XS // 2]
                    )

            # SWDGE ring: everything else, in need-order
            wv_all = cpool.tile([128, CCH * 512], bf, tag="wv", name="wv_all")
            nc.gpsimd.dma_start(wv_all[:, : 4 * 512], wv[:, : 4 * 512])
            nc.gpsimd.dma_start(wv_all[:, 4 * 512 :], wv[:, 4 * 512 :])
            mp_sb = cpool.tile([128, 1408], bf, tag="mpk", name="mp_sb")
            nc.gpsimd.dma_start(mp_sb[:], mpk[:, :])
            mask_sb = [
                mp_sb[:, MASK_OFF[d] : MASK_OFF[d] + MASK_W[d]] for d in range(4)
            ]
            ident_sb = mp_sb[:, 1280:1408]
            bias_sb = cpool.tile([128, 16], f32, tag="bias", name="bias_sb")
            nc.gpsimd.dma_start(bias_sb[:], bias[:, :])
            wpr_all = cpool.tile([128, 4 * 1024], bf, tag="wpr", name="wpr_all")
            nc.gpsimd.dma_start(wpr_all[:, : 2 * 1024], wpr[:, : 2 * 1024])
            nc.gpsimd.dma_start(wpr_all[:, 2 * 1024 :], wpr[:, 2 * 1024 :])
            bqk_sb = bias_sb[:, 0:8]
            bpr_sb = bias_sb[:, 8:16]

            ones_sb = cpool.tile([128, 1], f32, tag="ones", name="ones")
            nc.vector.memset(ones_sb[:], 1.0)

            # persistent K^T tiles per (head-pair, block) and V tiles per tk chunk
            kT = [[None] * NB for _ in range(HP)]
            vt = [None] * TCH
            qT_store = [[None] * 4 for _ in range(NB)]

            def qk_group(j, t):
                ps = papool.tile([128, BLK], f32, tag="pa", name="pa")
                for c in range(CCH):
                    nc.tensor.matmul(
                        ps[:],
                        lhsT=wqk_all[:, c * 1024 + t * 128 : c * 1024 + (t + 1) * 128],
                        rhs=xts(j, c),
                        start=(c == 0),
                        stop=(c == CCH - 1),
                    )
                if t < 4:
                    dst = qtpool.tile([128, BLK], bf, tag=f"qT{t}", name=f"qT{t}")
                    qT_store[j][t] = dst
                else:
                    dst = kvpool.tile(
                        [128, BLK], bf, tag=f"kT{t - 4}_{j}", name=f"kT{t - 4}_{j}"
                    )
                    kT[t - 4][j] = dst
                nc.vector.tensor_scalar_add(dst[:], ps[:], bqk_sb[:, t : t + 1])

            def v_group(j, tcl):
                tci = 4 * j + tcl
                ps = papool.tile([128, BLK], f32, tag="pa", name="pa")
                for c in range(CCH):
                    nc.tensor.matmul(
                        ps[:],
                        lhsT=xts(j, c)[:, tcl * 128 : (tcl + 1) * 128],
                        rhs=wv_all[:, c * 512 : (c + 1) * 512],
                        start=(c == 0),
                        stop=(c == CCH - 1),
                    )
                v_ = kvpool.tile([128, H, 65], bf, tag=f"v{tci}", name=f"v{tci}")
                vt[tci] = v_
                nc.vector.tensor_copy(
                    v_[:, :, 64], ones_sb[:, 0:1].to_broadcast([128, H])
                )
                nc.vector.tensor_copy(
                    v_[:, :, :64], ps[:].rearrange("p (h d) -> p h d", d=64)
                )

            def qkv_thunks(j):
                return [lambda t=t: qk_group(j, t) for t in range(8)] + [
                    lambda tcl=tcl: v_group(j, tcl) for tcl in range(4)
                ]

            # prologue: block 0 QKV
            for th in qkv_thunks(0):
                th()

            for j in range(NB):
                nchunks = 4 * j + 4
                pending = qkv_thunks(j + 1) if j + 1 < NB else []
                emitted = 0
                total_iters = HP * nchunks
                it = 0

                oT = [
                    otpool.tile([128, BLK], bf, tag=f"oT{hp}", name=f"oT{hp}")
                    for hp in range(HP)
                ]
                qT = qT_store[j]
                for hp in range(HP):
                    o_ps = [
                        opool.tile([65, BLK], f32, tag="o", name="o_ps")
                        for _ in range(2)
                    ]
                    prev = None
                    for tci in range(nchunks):
                        d = tci - 4 * j
                        c0 = 0 if d < 0 else SCORE_C0[d]
                        st = stpool.tile([128, 2 * BLK], f32, tag="st", name="st")
                        for s in range(2):
                            sb = s * BLK
                            kslc = kT[hp][tci // 4][
                                64 * s : 64 * s + 64,
                                (tci % 4) * 128 : (tci % 4 + 1) * 128,
                            ]
                            if d >= 0:
                                nc.tensor.matmul(
                                    st[:, sb : sb + MASK_W[d]],
                                    lhsT=ident_sb,
                                    rhs=mask_sb[d],
                                    start=True,
                                    stop=False,
                                )
                            nc.tensor.matmul(
                                st[:, sb + c0 : sb + BLK],
                                lhsT=kslc,
                                rhs=qT[hp][64 * s : 64 * s + 64, c0:],
                                start=(d < 0),
                                stop=True,
                                tile_position=(64 * s, 0),
                            )
                        pt = ptpool.tile([128, 2 * BLK], bf, tag="pT", name="pT")
                        nc.scalar.activation(pt[:], st[:], Exp, scale=0.125)
                        if prev is not None:
                            pt_p, tcp, c0p = prev
                            for s in range(2):
                                nc.tensor.matmul(
                                    o_ps[s][:, c0p:],
                                    lhsT=vt[tcp][:, 2 * hp + s, :],
                                    rhs=pt_p[:, s * BLK + c0p : (s + 1) * BLK],
                                    start=(tcp == 0),
                                    stop=(tcp == nchunks - 1),
                                )
                        prev = (pt, tci, c0)
                        # interleave next block's QKV groups into the PE stream
                        it += 1
                        want = (it * len(pending)) // total_iters if pending else 0
                        while emitted < want:
                            pending[emitted]()
                            emitted += 1
                    pt_p, tcp, c0p = prev
                    for s in range(2):
                        nc.tensor.matmul(
                            o_ps[s][:, c0p:],
                            lhsT=vt[tcp][:, 2 * hp + s, :],
                            rhs=pt_p[:, s * BLK + c0p : (s + 1) * BLK],
                            start=(tcp == 0),
                            stop=(tcp == nchunks - 1),
                        )
                    for s in range(2):
                        o_sb = osbpool.tile([65, BLK], f32, tag="osb", name="o_sb")
                        nc.vector.tensor_copy(o_sb[:], o_ps[s][:])
                        rc = rpool.tile([1, BLK], f32, tag="rcp", name="rcp")
                        nc.vector.reciprocal(rc[:], o_sb[64:65, :])
                        bc = rpool.tile([64, BLK], f32, tag="bc", name="bc")
                        nc.gpsimd.partition_broadcast(bc[:], rc[:])
                        nc.vector.tensor_tensor(
                            oT[hp][64 * s : 64 * s + 64, :],
                            o_sb[:64, :],
                            bc[:],
                            Mult,
                        )
                while emitted < len(pending):
                    pending[emitted]()
                    emitted += 1

                # ---------------- output projection for block j ----------------
                ysb = ypool.tile([128, 8 * BLK], bf, tag="y", name="ysb")
                for t in range(8):
                    ps = papool.tile([128, BLK], f32, tag="pa", name="pa")
                    for cp in range(4):
                        nc.tensor.matmul(
                            ps[:],
                            lhsT=wpr_all[
                                :, cp * 1024 + t * 128 : cp * 1024 + (t + 1) * 128
                            ],
                            rhs=oT[cp][:],
                            start=(cp == 0),
                            stop=(cp == 3),
                        )
                    nc.vector.tensor_scalar_add(
                        ysb[:, t * BLK : (t + 1) * BLK], ps[:], bpr_sb[:, t : t + 1]
                    )
                nc.gpsimd.dma_start(
                    yT[:, j * 8 * BLK : (j + 1) * 8 * BLK], ysb[:]
                )

    nc.compile()
    return nc


def _host_inputs(x, W_attn, b_attn, W_proj, b_proj):
    """Build the 8 per-core input maps."""
    x = np.asarray(x, dtype=np.float32)
    W_attn = np.asarray(W_attn, dtype=np.float32)
    b_attn = np.asarray(b_attn, dtype=np.float32)
    W_proj = np.asarray(W_proj, dtype=np.float32)
    b_proj = np.asarray(b_proj, dtype=np.float32)

    p = np.arange(128)[:, None]
    maskpack = np.zeros((128, 1408), dtype=np.float32)
    for d in range(4):
        f = np.arange(MASK_W[d])[None, :]
        maskpack[:, MASK_OFF[d] : MASK_OFF[d] + MASK_W[d]] = np.where(
            f >= 128 * d + p, 0.0, NEG
        )
    maskpack[:, 1280:1408] = np.eye(128, dtype=np.float32)
    maskpack = maskpack.astype(BF16)

    in_maps = []
    for core in range(8):
        b = core // 2
        hh = core % 2
        cs = hh * 512
        wq = W_attn[:, cs : cs + 512]
        wk = W_attn[:, C + cs : C + cs + 512]
        wv_ = W_attn[:, 2 * C + cs : 2 * C + cs + 512]
        bq = b_attn[cs : cs + 512]
        bk = b_attn[C + cs : C + cs + 512]
        bv = b_attn[2 * C + cs : 2 * C + cs + 512]
        wpr_ = W_proj[cs : cs + 512, :]
        bpr_eff = bv @ wpr_ + (b_proj if hh == 0 else 0.0)
        biases = np.concatenate(
            [
                np.concatenate([bq, bk]).reshape(8, 128).T,
                bpr_eff.astype(np.float32).reshape(8, 128).T,
            ],
            axis=1,
        )
        # xT packed: [128, (j, c, f)] with xT_pack[p, j, c, f] = x[b][j*BLK+f, c*128+p]
        xt = x[b].T.reshape(CCH, 128, NB, BLK)          # [c, p, j, f]
        xt_pack = np.ascontiguousarray(xt.transpose(1, 2, 0, 3)).reshape(
            128, NB * CCH * BLK
        )
        # weights packed: [128, (c, n)] with w_pack[p, c, n] = w[c*128+p, n]
        wqk_n = np.concatenate([wq, wk], axis=1)        # [C, 1024]
        wqk_pack = np.ascontiguousarray(
            wqk_n.reshape(CCH, 128, 1024).transpose(1, 0, 2)
        ).reshape(128, CCH * 1024)
        wv_pack = np.ascontiguousarray(
            wv_.reshape(CCH, 128, 512).transpose(1, 0, 2)
        ).reshape(128, CCH * 512)
        wpr_pack = np.ascontiguousarray(
            wpr_.reshape(4, 128, 1024).transpose(1, 0, 2)
        ).reshape(128, 4 * 1024)
        in_maps.append(
            {
                "xT": xt_pack.astype(BF16),
                "wqk": wqk_pack.astype(BF16),
                "wv": wv_pack.astype(BF16),
                "wproj": wpr_pack.astype(BF16),
                "maskpack": maskpack,
                "biases": np.ascontiguousarray(biases.astype(np.float32)),
            }
        )
    return in_maps


def run(x, W_attn, b_attn, W_proj, b_proj, trace=False):
    from concourse.bass_utils import run_bass_kernel_spmd

    if "nc" not in _CACHE:
        _CACHE["nc"] = _build_nc()
    nc = _CACHE["nc"]
    in_maps = _host_inputs(x, W_attn, b_attn, W_proj, b_proj)
    res = run_bass_kernel_spmd(nc, in_maps, core_ids=list(range(8)), trace=False)
    y = np.empty((B, T, C), dtype=np.float32)
    for b in range(B):
        # yT packed per core: [128, (j, t, f)] = y^T[t*128+p, j*BLK+f]
        acc = None
        for hh in range(2):
            yt = res.results[2 * b + hh]["yT"].astype(np.float32)
            yt = yt.reshape(128, NB, 8, BLK).transpose(2, 0, 1, 3).reshape(C, T)
            acc = yt if acc is None else acc + yt
        y[b] = acc.T
    return y, res


def kernel(x, W_attn, b_attn, W_proj, b_proj):
    y, _ = run(x, W_attn, b_attn, W_proj, b_proj, trace=False)
    return y


def make_timed_runner(in_maps=None, nc=None):
    """Build a non-donating jitted SPMD callable with device-resident inputs.

    Returns fn(n) -> wall seconds to execute the kernel n times back-to-back
    (async dispatch, single block at the end). Differential timing
    (wall(n) - wall(1)) / (n - 1) estimates per-execution device time plus
    a per-call dispatch overhead (~400us on this axon setup; calibrate with
    a trivial kernel and subtract).
    """
    import jax
    import numpy as _np
    import concourse.mybir as mybir
    from concourse import bass2jax
    from jax.experimental.shard_map import shard_map
    from jax.sharding import Mesh, PartitionSpec, NamedSharding

    if nc is None:
        if "nc" not in _CACHE:
            _CACHE["nc"] = _build_nc()
        nc = _CACHE["nc"]

    bass2jax.install_neuronx_cc_hook()
    n_cores = 8

    partition_name = nc.partition_id_tensor.name if nc.partition_id_tensor else None
    in_names, out_names, out_avals, zero_outs = [], [], [], []
    for alloc in nc.m.functions[0].allocations:
        if not isinstance(alloc, mybir.MemoryLocationSet):
            continue
        name = alloc.memorylocations[0].name
        if alloc.kind == "ExternalInput":
            if name != partition_name:
                in_names.append(name)
        elif alloc.kind == "ExternalOutput":
            out_names.append(name)
            shape = tuple(alloc.tensor_shape)
            dtype = mybir.dt.np(alloc.dtype)
            out_avals.append(jax.core.ShapedArray(shape, dtype))
            zero_outs.append(_np.zeros(shape, dtype))
    n_params = len(in_names)
    all_names = in_names + out_names
    if partition_name is not None:
        all_names = all_names + [partition_name]

    def _body(*args):
        operands = list(args)
        if partition_name is not None:
            operands.append(bass2jax.partition_id_tensor())
        outs = bass2jax._bass_exec_p.bind(
            *operands,
            out_avals=tuple(out_avals),
            in_names=tuple(all_names),
            out_names=tuple(out_names),
            lowering_input_output_aliases=(),
            sim_require_finite=True,
            sim_require_nnan=True,
            nc=nc,
        )
        return tuple(outs)

    devices = jax.devices()[:n_cores]
    mesh = Mesh(_np.asarray(devices), ("core",))
    spec = PartitionSpec("core")
    sharded = jax.jit(
        shard_map(
            _body,
            mesh=mesh,
            in_specs=(spec,) * (n_params + len(out_names)),
            out_specs=(spec,) * len(out_names),
            check_rep=False,
        ),
        keep_unused=True,
    )
    sh = NamedSharding(mesh, spec)
    dev_args = [
        jax.device_put(
            _np.concatenate([_np.asarray(in_maps[c][nm]) for c in range(n_cores)], 0),
            sh,
        )
        for nm in in_names
    ] + [
        jax.device_put(
            _np.zeros((n_cores * z.shape[0], *z.shape[1:]), z.dtype), sh
        )
        for z in zero_outs
    ]

    import time as _time

    def timed(n):
        out = None
        t0 = _time.perf_counter()
        for _ in range(n):
            out = sharded(*dev_args)
        jax.block_until_ready(out)
        return _time.perf_counter() - t0

    return timed


# revision 20
# speedup vs baseline: 2.8509x; 1.7268x over previous
"""Causal self-attention (B=4, T=2048, C=1024, 16 heads) on 8 trn2 NeuronCores.

Sharding: core i handles batch b=i//2 and head-half hh=i%2 (8 of 16 heads).
Each core computes its 8 heads' attention output projected through its slice
of W_proj rows (a partial sum of y); host adds the two head-half partials per
batch and transposes back.

Layout strategy (matmul operands in bf16, fp32 PSUM accumulation):
  - host pre-transposes x[b] -> xT [C, T]; xT and all weights live in SBUF
    for the whole kernel, loaded by a handful of large DMAs
  - qk^T = W_qk.T @ x (via lhsT=W_qk chunks, rhs=xT chunks): [qk_cols, T]
  - V natural [T, vcols] (via lhsT=xT chunk, rhs=W_v), with a ones column
    per head so the PV matmul also produces the softmax denominator
  - S^T[tk, tq] = K_h @ Q_h^T via lhsT=K^T cols, rhs=Q^T (two heads packed
    into the 128-row PE array with tile_position row groups); both heads of
    a pair share one [128, 1024] PSUM tile so exp runs as a single ACT op
  - causal mask added in PSUM by an identity-weight matmul of a -1e30 mask
  - P^T = exp(S^T/8) on ScalarE (masked entries underflow to exactly 0)
  - O^T[65, tq] accumulates lhsT=V_ext[tk,65], rhs=P^T; row 64 = sum(exp);
    O copies to SBUF immediately (frees the PSUM bank), then reciprocal of
    row 64 on DVE, gpsimd partition_broadcast, multiply on DVE
  - next block's QKV matmul groups are interleaved into the attention
    instruction stream so the in-order PE queue has filler work while
    ScalarE computes exp
  - y^T = W_proj.T @ attn_out^T accumulated over head pairs; bias per
    partition; one output DMA per block
"""

import sys

sys.path.insert(0, "/opt/trn_rl_repo")

import numpy as np
import ml_dtypes

BF16 = ml_dtypes.bfloat16

B, T, C = 4, 2048, 1024
NHEAD_GLOBAL = 16
D = 64
H = 8                    # local heads per core
HP = H // 2              # head pairs
NB = 4                   # tq blocks
BLK = T // NB            # 512
CCH = C // 128           # 8 contraction chunks
TCH = T // 128           # 16 tk chunks
NEG = -1.0e30

SCORE_C0 = [0, 128, 256, 384]     # scores/pv matmul col start per diagonal pos d

_CACHE = {}


def _build_nc():
    import concourse.bass as bass  # noqa: F401
    import concourse.mybir as mybir
    import concourse.tile as tile
    from concourse import bacc

    f32 = mybir.dt.float32
    bf = mybir.dt.bfloat16

    nc = bacc.Bacc("TRN2", target_bir_lowering=False, debug=False)

    xT = nc.dram_tensor("xT", [128, NB * CCH * BLK], bf, kind="ExternalInput").ap()
    wqk = nc.dram_tensor("wqk", [128, CCH * 1024], bf, kind="ExternalInput").ap()
    wv = nc.dram_tensor("wv", [128, CCH * 512], bf, kind="ExternalInput").ap()
    wpr = nc.dram_tensor("wproj", [128, 4 * 1024], bf, kind="ExternalInput").ap()
    mpk = nc.dram_tensor("maskpack", [128, 256], bf, kind="ExternalInput").ap()
    bias = nc.dram_tensor("biases", [128, 16], f32, kind="ExternalInput").ap()
    yT = nc.dram_tensor("yT", [128, NB * 8 * BLK], bf, kind="ExternalOutput").ap()

    Exp = mybir.ActivationFunctionType.Exp
    Mult = mybir.AluOpType.mult

    with tile.TileContext(nc) as tc:
        with (
            tc.tile_pool(name="const", bufs=1) as cpool,
            tc.tile_pool(name="kv", bufs=1) as kvpool,
            tc.tile_pool(name="qt", bufs=2) as qtpool,
            tc.tile_pool(name="pt", bufs=6) as ptpool,
            tc.tile_pool(name="ot", bufs=2) as otpool,
            tc.tile_pool(name="osb", bufs=4) as osbpool,
            tc.tile_pool(name="ysb", bufs=2) as ypool,
            tc.tile_pool(name="rcp", bufs=3) as rpool,
            tc.tile_pool(name="pa_ps", bufs=2, space="PSUM") as papool,
            tc.tile_pool(name="st_ps", bufs=2, space="PSUM") as stpool,
            tc.tile_pool(name="o_ps", bufs=2, space="PSUM") as opool,
        ):
            # ---- resident inputs: flat 2D DMAs in dependency order ----
            xt_all = cpool.tile([128, NB * CCH * BLK], bf, tag="xt", name="xt_all")
            wqk_all = cpool.tile([128, CCH * 1024], bf, tag="wqk", name="wqk_all")
            XS = CCH * BLK

            def xts(jj, c):
                off = (jj * CCH + c) * BLK
                return xt_all[:, off : off + BLK]

            # SP ring: interleave block-0 x chunks with wqk chunks so the
            # first qk matmuls start after ~0.5MB has landed, then stream
            # the remaining x blocks behind compute.
            for c in range(CCH):
                nc.sync.dma_start(xts(0, c), xT[:, c * BLK : (c + 1) * BLK])
                nc.sync.dma_start(
                    wqk_all[:, c * 1024 : (c + 1) * 1024],
                    wqk[:, c * 1024 : (c + 1) * 1024],
                )
            for jj in range(1, NB):
                for ch in range(2):
                    lo = jj * XS + ch * (XS // 2)
                    nc.sync.dma_start(
                        xt_all[:, lo : lo + XS // 2], xT[:, lo : lo + XS // 2]
                    )

            # SWDGE ring: everything else, in need-order
            wv_all = cpool.tile([128, CCH * 512], bf, tag="wv", name="wv_all")
            nc.gpsimd.dma_start(wv_all[:, : 4 * 512], wv[:, : 4 * 512])
            nc.gpsimd.dma_start(wv_all[:, 4 * 512 :], wv[:, 4 * 512 :])
            mp_sb = cpool.tile([128, 256], bf, tag="mpk", name="mp_sb")
            nc.gpsimd.dma_start(mp_sb[:], mpk[:, :])
            mask128 = mp_sb[:, 0:128]
            ident_sb = mp_sb[:, 128:256]
            bias_sb = cpool.tile([128, 16], f32, tag="bias", name="bias_sb")
            nc.gpsimd.dma_start(bias_sb[:], bias[:, :])
            wpr_all = cpool.tile([128, 4 * 1024], bf, tag="wpr", name="wpr_all")
            nc.gpsimd.dma_start(wpr_all[:, : 2 * 1024], wpr[:, : 2 * 1024])
            nc.gpsimd.dma_start(wpr_all[:, 2 * 1024 :], wpr[:, 2 * 1024 :])
            bqk_sb = bias_sb[:, 0:8]
            bpr_sb = bias_sb[:, 8:16]

            ones_sb = cpool.tile([128, 1], f32, tag="ones", name="ones")
            nc.vector.memset(ones_sb[:], 1.0)

            # persistent K^T tiles per (head-pair, block) and V tiles per tk chunk
            kT = [[None] * NB for _ in range(HP)]
            vt = [None] * TCH
            qT_store = [[None] * 4 for _ in range(NB)]

            def qk_group(j, t):
                ps = papool.tile([128, BLK], f32, tag="pa", name="pa")
                for c in range(CCH):
                    nc.tensor.matmul(
                        ps[:],
                        lhsT=wqk_all[:, c * 1024 + t * 128 : c * 1024 + (t + 1) * 128],
                        rhs=xts(j, c),
                        start=(c == 0),
                        stop=(c == CCH - 1),
                    )
                if t < 4:
                    dst = qtpool.tile([128, BLK], bf, tag=f"qT{t}", name=f"qT{t}")
                    qT_store[j][t] = dst
                else:
                    dst = kvpool.tile(
                        [128, BLK], bf, tag=f"kT{t - 4}_{j}", name=f"kT{t - 4}_{j}"
                    )
                    kT[t - 4][j] = dst
                nc.vector.tensor_scalar_add(dst[:], ps[:], bqk_sb[:, t : t + 1])

            def v_group(j, tcl):
                tci = 4 * j + tcl
                ps = papool.tile([128, BLK], f32, tag="pa", name="pa")
                for c in range(CCH):
                    nc.tensor.matmul(
                        ps[:],
                        lhsT=xts(j, c)[:, tcl * 128 : (tcl + 1) * 128],
                        rhs=wv_all[:, c * 512 : (c + 1) * 512],
                        start=(c == 0),
                        stop=(c == CCH - 1),
                    )
                v_ = kvpool.tile([128, H, 65], bf, tag=f"v{tci}", name=f"v{tci}")
                vt[tci] = v_
                nc.vector.tensor_copy(
                    v_[:, :, 64], ones_sb[:, 0:1].to_broadcast([128, H])
                )
                nc.vector.tensor_copy(
                    v_[:, :, :64], ps[:].rearrange("p (h d) -> p h d", d=64)
                )

            def qkv_thunks(j):
                return [lambda t=t: qk_group(j, t) for t in range(8)] + [
                    lambda tcl=tcl: v_group(j, tcl) for tcl in range(4)
                ]

            # prologue: block 0 QKV
            for th in qkv_thunks(0):
                th()

            for j in range(NB):
                nchunks = 4 * j + 4
                pending = qkv_thunks(j + 1) if j + 1 < NB else []
                emitted = 0
                total_iters = HP * nchunks
                it = 0

                oT = [
                    otpool.tile([128, BLK], bf, tag=f"oT{hp}", name=f"oT{hp}")
                    for hp in range(HP)
                ]
                qT = qT_store[j]
                for hp in range(HP):
                    o_ps = [
                        opool.tile([65, BLK], f32, tag="o", name="o_ps")
                        for _ in range(2)
                    ]
                    prev = None
                    for tci in range(nchunks):
                        d = tci - 4 * j
                        c0 = 0 if d < 0 else SCORE_C0[d]
                        st = stpool.tile([128, 2 * BLK], f32, tag="st", name="st")
                        for s in range(2):
                            sb = s * BLK
                            kslc = kT[hp][tci // 4][
                                64 * s : 64 * s + 64,
                                (tci % 4) * 128 : (tci % 4 + 1) * 128,
                            ]
                            if d >= 0:
                                # causal triangle: cols [c0, c0+128) of this
                                # chunk get the -1e30 upper-triangular mask
                                nc.tensor.matmul(
                                    st[:, sb + c0 : sb + c0 + 128],
                                    lhsT=ident_sb,
                                    rhs=mask128,
                                    start=True,
                                    stop=False,
                                )
                            nc.tensor.matmul(
                                st[:, sb + c0 : sb + BLK],
                                lhsT=kslc,
                                rhs=qT[hp][64 * s : 64 * s + 64, c0:],
                                start=(d < 0),
                                stop=True,
                                tile_position=(64 * s, 0),
                            )
                        pt = ptpool.tile([128, 2 * BLK], bf, tag="pT", name="pT")
                        if d >= 2:
                            # most of the tile is masked dead; exp only the
                            # live [c0, BLK) span of each head
                            for s in range(2):
                                sb = s * BLK
                                nc.scalar.activation(
                                    pt[:, sb + c0 : sb + BLK],
                                    st[:, sb + c0 : sb + BLK],
                                    Exp,
                                    scale=0.125,
                                )
                        else:
                            nc.scalar.activation(pt[:], st[:], Exp, scale=0.125)
                        if prev is not None:
                            pt_p, tcp, c0p = prev
                            for s in range(2):
                                nc.tensor.matmul(
                                    o_ps[s][:, c0p:],
                                    lhsT=vt[tcp][:, 2 * hp + s, :],
                                    rhs=pt_p[:, s * BLK + c0p : (s + 1) * BLK],
                                    start=(tcp == 0),
                                    stop=(tcp == nchunks - 1),
                                )
                        prev = (pt, tci, c0)
                        # interleave next block's QKV groups into the PE stream
                        it += 1
                        want = (it * len(pending)) // total_iters if pending else 0
                        while emitted < want:
                            pending[emitted]()
                            emitted += 1
                    pt_p, tcp, c0p = prev
                    for s in range(2):
                        nc.tensor.matmul(
                            o_ps[s][:, c0p:],
                            lhsT=vt[tcp][:, 2 * hp + s, :],
                            rhs=pt_p[:, s * BLK + c0p : (s + 1) * BLK],
                            start=(tcp == 0),
                            stop=(tcp == nchunks - 1),
                        )
                    for s in range(2):
                        o_sb = osbpool.tile([65, BLK], f32, tag="osb", name="o_sb")
                        nc.vector.tensor_copy(o_sb[:], o_ps[s][:])
                        rc = rpool.tile([1, BLK], f32, tag="rcp", name="rcp")
                        nc.vector.reciprocal(rc[:], o_sb[64:65, :])
                        bc = rpool.tile([64, BLK], f32, tag="bc", name="bc")
                        nc.gpsimd.partition_broadcast(bc[:], rc[:])
                        nc.vector.tensor_tensor(
                            oT[hp][64 * s : 64 * s + 64, :],
                            o_sb[:64, :],
                            bc[:],
                            Mult,
                        )
                while emitted < len(pending):
                    pending[emitted]()
                    emitted += 1

                # ---------------- output projection for block j ----------------
                ysb = ypool.tile([128, 8 * BLK], bf, tag="y", name="ysb")
                for t in range(8):
                    ps = papool.tile([128, BLK], f32, tag="pa", name="pa")
                    for cp in range(4):
                        nc.tensor.matmul(
                            ps[:],
                            lhsT=wpr_all[
                                :, cp * 1024 + t * 128 : cp * 1024 + (t + 1) * 128
                            ],
                            rhs=oT[cp][:],
                            start=(cp == 0),
                            stop=(cp == 3),
                        )
                    nc.vector.tensor_scalar_add(
                        ysb[:, t * BLK : (t + 1) * BLK], ps[:], bpr_sb[:, t : t + 1]
                    )
                nc.gpsimd.dma_start(
                    yT[:, j * 8 * BLK : (j + 1) * 8 * BLK], ysb[:]
                )

    nc.compile()
    return nc


def _host_inputs(x, W_attn, b_attn, W_proj, b_proj):
    """Build the 8 per-core input maps."""
    x = np.asarray(x, dtype=np.float32)
    W_attn = np.asarray(W_attn, dtype=np.float32)
    b_attn = np.asarray(b_attn, dtype=np.float32)
    W_proj = np.asarray(W_proj, dtype=np.float32)
    b_proj = np.asarray(b_proj, dtype=np.float32)

    p = np.arange(128)[:, None]
    g = np.arange(128)[None, :]
    maskpack = np.zeros((128, 256), dtype=np.float32)
    maskpack[:, 0:128] = np.where(g >= p, 0.0, NEG)
    maskpack[:, 128:256] = np.eye(128, dtype=np.float32)
    maskpack = maskpack.astype(BF16)

    in_maps = []
    for core in range(8):
        b = core // 2
        hh = core % 2
        cs = hh * 512
        wq = W_attn[:, cs : cs + 512]
        wk = W_attn[:, C + cs : C + cs + 512]
        wv_ = W_attn[:, 2 * C + cs : 2 * C + cs + 512]
        bq = b_attn[cs : cs + 512]
        bk = b_attn[C + cs : C + cs + 512]
        bv = b_attn[2 * C + cs : 2 * C + cs + 512]
        wpr_ = W_proj[cs : cs + 512, :]
        bpr_eff = bv @ wpr_ + (b_proj if hh == 0 else 0.0)
        biases = np.concatenate(
            [
                np.concatenate([bq, bk]).reshape(8, 128).T,
                bpr_eff.astype(np.float32).reshape(8, 128).T,
            ],
            axis=1,
        )
        # xT packed: [128, (j, c, f)] with xT_pack[p, j, c, f] = x[b][j*BLK+f, c*128+p]
        xt = x[b].T.reshape(CCH, 128, NB, BLK)          # [c, p, j, f]
        xt_pack = np.ascontiguousarray(xt.transpose(1, 2, 0, 3)).reshape(
            128, NB * CCH * BLK
        )
        # weights packed: [128, (c, n)] with w_pack[p, c, n] = w[c*128+p, n]
        wqk_n = np.concatenate([wq, wk], axis=1)        # [C, 1024]
        wqk_pack = np.ascontiguousarray(
            wqk_n.reshape(CCH, 128, 1024).transpose(1, 0, 2)
        ).reshape(128, CCH * 1024)
        wv_pack = np.ascontiguousarray(
            wv_.reshape(CCH, 128, 512).transpose(1, 0, 2)
        ).reshape(128, CCH * 512)
        wpr_pack = np.ascontiguousarray(
            wpr_.reshape(4, 128, 1024).transpose(1, 0, 2)
        ).reshape(128, 4 * 1024)
        in_maps.append(
            {
                "xT": xt_pack.astype(BF16),
                "wqk": wqk_pack.astype(BF16),
                "wv": wv_pack.astype(BF16),
                "wproj": wpr_pack.astype(BF16),
                "maskpack": maskpack,
                "biases": np.ascontiguousarray(biases.astype(np.float32)),
            }
        )
    return in_maps


def run(x, W_attn, b_attn, W_proj, b_proj, trace=False):
    from concourse.bass_utils import run_bass_kernel_spmd

    if "nc" not in _CACHE:
        _CACHE["nc"] = _build_nc()
    nc = _CACHE["nc"]
    in_maps = _host_inputs(x, W_attn, b_attn, W_proj, b_proj)
    res = run_bass_kernel_spmd(nc, in_maps, core_ids=list(range(8)), trace=False)
    y = np.empty((B, T, C), dtype=np.float32)
    for b in range(B):
        # yT packed per core: [128, (j, t, f)] = y^T[t*128+p, j*BLK+f]
        acc = None
        for hh in range(2):
            yt = res.results[2 * b + hh]["yT"].astype(np.float32)
            yt = yt.reshape(128, NB, 8, BLK).transpose(2, 0, 1, 3).reshape(C, T)
            acc = yt if acc is None else acc + yt
        y[b] = acc.T
    return y, res


def kernel(x, W_attn, b_attn, W_proj, b_proj):
    y, _ = run(x, W_attn, b_attn, W_proj, b_proj, trace=False)
    return y


def make_timed_runner(in_maps=None, nc=None):
    """Build a non-donating jitted SPMD callable with device-resident inputs.

    Returns fn(n) -> wall seconds to execute the kernel n times back-to-back
    (async dispatch, single block at the end). Differential timing
    (wall(n) - wall(1)) / (n - 1) estimates per-execution device time plus
    a per-call dispatch overhead (~400us on this axon setup; calibrate with
    a trivial kernel and subtract).
    """
    import jax
    import numpy as _np
    import concourse.mybir as mybir
    from concourse import bass2jax
    from jax.experimental.shard_map import shard_map
    from jax.sharding import Mesh, PartitionSpec, NamedSharding

    if nc is None:
        if "nc" not in _CACHE:
            _CACHE["nc"] = _build_nc()
        nc = _CACHE["nc"]

    bass2jax.install_neuronx_cc_hook()
    n_cores = 8

    partition_name = nc.partition_id_tensor.name if nc.partition_id_tensor else None
    in_names, out_names, out_avals, zero_outs = [], [], [], []
    for alloc in nc.m.functions[0].allocations:
        if not isinstance(alloc, mybir.MemoryLocationSet):
            continue
        name = alloc.memorylocations[0].name
        if alloc.kind == "ExternalInput":
            if name != partition_name:
                in_names.append(name)
        elif alloc.kind == "ExternalOutput":
            out_names.append(name)
            shape = tuple(alloc.tensor_shape)
            dtype = mybir.dt.np(alloc.dtype)
            out_avals.append(jax.core.ShapedArray(shape, dtype))
            zero_outs.append(_np.zeros(shape, dtype))
    n_params = len(in_names)
    all_names = in_names + out_names
    if partition_name is not None:
        all_names = all_names + [partition_name]

    def _body(*args):
        operands = list(args)
        if partition_name is not None:
            operands.append(bass2jax.partition_id_tensor())
        outs = bass2jax._bass_exec_p.bind(
            *operands,
            out_avals=tuple(out_avals),
            in_names=tuple(all_names),
            out_names=tuple(out_names),
            lowering_input_output_aliases=(),
            sim_require_finite=True,
            sim_require_nnan=True,
            nc=nc,
        )
        return tuple(outs)

    devices = jax.devices()[:n_cores]
    mesh = Mesh(_np.asarray(devices), ("core",))
    spec = PartitionSpec("core")
    sharded = jax.jit(
        shard_map(
            _body,
            mesh=mesh,
            in_specs=(spec,) * (n_params + len(out_names)),
            out_specs=(spec,) * len(out_names),
            check_rep=False,
        ),
        keep_unused=True,
    )
    sh = NamedSharding(mesh, spec)
    dev_args = [
        jax.device_put(
            _np.concatenate([_np.asarray(in_maps[c][nm]) for c in range(n_cores)], 0),
            sh,
        )
        for nm in in_names
    ] + [
        jax.device_put(
            _np.zeros((n_cores * z.shape[0], *z.shape[1:]), z.dtype), sh
        )
        for z in zero_outs
    ]

    import time as _time

    def timed(n):
        out = None
        t0 = _time.perf_counter()
        for _ in range(n):
            out = sharded(*dev_args)
        jax.block_until_ready(out)
        return _time.perf_counter() - t0

    return timed


# revision 21
# speedup vs baseline: 2.9388x; 1.0309x over previous
"""Causal self-attention (B=4, T=2048, C=1024, 16 heads) on 8 trn2 NeuronCores.

Sharding: core i handles batch b=i//2 and head-half hh=i%2 (8 of 16 heads).
Each core computes its 8 heads' attention output projected through its slice
of W_proj rows (a partial sum of y); host adds the two head-half partials per
batch and transposes back.

Layout strategy (matmul operands in bf16, fp32 PSUM accumulation):
  - host pre-transposes x[b] -> xT [C, T]; xT and all weights live in SBUF
    for the whole kernel, loaded by a handful of large DMAs
  - qk^T = W_qk.T @ x (via lhsT=W_qk chunks, rhs=xT chunks): [qk_cols, T]
  - V natural [T, vcols] (via lhsT=xT chunk, rhs=W_v), with a ones column
    per head so the PV matmul also produces the softmax denominator
  - S^T[tk, tq] = K_h @ Q_h^T via lhsT=K^T cols, rhs=Q^T (two heads packed
    into the 128-row PE array with tile_position row groups); both heads of
    a pair share one [128, 1024] PSUM tile so exp runs as a single ACT op
  - causal mask added in PSUM by an identity-weight matmul of a -1e30 mask
  - P^T = exp(S^T/8) on ScalarE (masked entries underflow to exactly 0)
  - O^T[65, tq] accumulates lhsT=V_ext[tk,65], rhs=P^T; row 64 = sum(exp);
    O copies to SBUF immediately (frees the PSUM bank), then reciprocal of
    row 64 on DVE, gpsimd partition_broadcast, multiply on DVE
  - next block's QKV matmul groups are interleaved into the attention
    instruction stream so the in-order PE queue has filler work while
    ScalarE computes exp
  - y^T = W_proj.T @ attn_out^T accumulated over head pairs; bias per
    partition; one output DMA per block
"""

import sys

sys.path.insert(0, "/opt/trn_rl_repo")

import numpy as np
import ml_dtypes

BF16 = ml_dtypes.bfloat16

B, T, C = 4, 2048, 1024
NHEAD_GLOBAL = 16
D = 64
H = 8                    # local heads per core
HP = H // 2              # head pairs
NB = 4                   # tq blocks
BLK = T // NB            # 512
CCH = C // 128           # 8 contraction chunks
TCH = T // 128           # 16 tk chunks
NEG = -1.0e30

SCORE_C0 = [0, 128, 256, 384]     # scores/pv matmul col start per diagonal pos d

_CACHE = {}


def _build_nc():
    import concourse.bass as bass  # noqa: F401
    import concourse.mybir as mybir
    import concourse.tile as tile
    from concourse import bacc

    f32 = mybir.dt.float32
    bf = mybir.dt.bfloat16

    nc = bacc.Bacc("TRN2", target_bir_lowering=False, debug=False)

    xT = nc.dram_tensor("xT", [128, NB * CCH * BLK], bf, kind="ExternalInput").ap()
    wqk = nc.dram_tensor("wqk", [128, CCH * 1024], bf, kind="ExternalInput").ap()
    wv = nc.dram_tensor("wv", [128, CCH * 512], bf, kind="ExternalInput").ap()
    wpr = nc.dram_tensor("wproj", [128, 4 * 1024], bf, kind="ExternalInput").ap()
    mpk = nc.dram_tensor("maskpack", [128, 256], bf, kind="ExternalInput").ap()
    bias = nc.dram_tensor("biases", [128, 16], f32, kind="ExternalInput").ap()
    yT = nc.dram_tensor("yT", [128, NB * 8 * BLK], bf, kind="ExternalOutput").ap()

    Exp = mybir.ActivationFunctionType.Exp
    Mult = mybir.AluOpType.mult

    with tile.TileContext(nc) as tc:
        with (
            tc.tile_pool(name="const", bufs=1) as cpool,
            tc.tile_pool(name="kv", bufs=1) as kvpool,
            tc.tile_pool(name="qt", bufs=2) as qtpool,
            tc.tile_pool(name="pt", bufs=6) as ptpool,
            tc.tile_pool(name="ot", bufs=2) as otpool,
            tc.tile_pool(name="osb", bufs=4) as osbpool,
            tc.tile_pool(name="ysb", bufs=2) as ypool,
            tc.tile_pool(name="rcp", bufs=3) as rpool,
            tc.tile_pool(name="pa_ps", bufs=2, space="PSUM") as papool,
            tc.tile_pool(name="st_ps", bufs=2, space="PSUM") as stpool,
            tc.tile_pool(name="o_ps", bufs=2, space="PSUM") as opool,
        ):
            # ---- resident inputs: flat 2D DMAs in dependency order ----
            xt_all = cpool.tile([128, NB * CCH * BLK], bf, tag="xt", name="xt_all")
            wqk_all = cpool.tile([128, CCH * 1024], bf, tag="wqk", name="wqk_all")
            XS = CCH * BLK

            def xts(jj, c):
                off = (jj * CCH + c) * BLK
                return xt_all[:, off : off + BLK]

            # SP ring: interleave block-0 x chunks with wqk chunks so the
            # first qk matmuls start after ~0.5MB has landed, then stream
            # the remaining x blocks behind compute.
            for c in range(CCH):
                nc.sync.dma_start(xts(0, c), xT[:, c * BLK : (c + 1) * BLK])
                nc.sync.dma_start(
                    wqk_all[:, c * 1024 : (c + 1) * 1024],
                    wqk[:, c * 1024 : (c + 1) * 1024],
                )
            for jj in range(1, NB):
                for ch in range(2):
                    lo = jj * XS + ch * (XS // 2)
                    nc.sync.dma_start(
                        xt_all[:, lo : lo + XS // 2], xT[:, lo : lo + XS // 2]
                    )

            # SWDGE ring: everything else, in need-order
            wv_all = cpool.tile([128, CCH * 512], bf, tag="wv", name="wv_all")
            nc.gpsimd.dma_start(wv_all[:, : 4 * 512], wv[:, : 4 * 512])
            nc.gpsimd.dma_start(wv_all[:, 4 * 512 :], wv[:, 4 * 512 :])
            mp_sb = cpool.tile([128, 256], bf, tag="mpk", name="mp_sb")
            nc.gpsimd.dma_start(mp_sb[:], mpk[:, :])
            mask128 = mp_sb[:, 0:128]
            ident_sb = mp_sb[:, 128:256]
            bias_sb = cpool.tile([128, 16], f32, tag="bias", name="bias_sb")
            nc.gpsimd.dma_start(bias_sb[:], bias[:, :])
            wpr_all = cpool.tile([128, 4 * 1024], bf, tag="wpr", name="wpr_all")
            nc.gpsimd.dma_start(wpr_all[:, : 2 * 1024], wpr[:, : 2 * 1024])
            nc.gpsimd.dma_start(wpr_all[:, 2 * 1024 :], wpr[:, 2 * 1024 :])
            bqk_sb = bias_sb[:, 0:8]
            bpr_sb = bias_sb[:, 8:16]

            ones_sb = cpool.tile([128, 1], f32, tag="ones", name="ones")
            nc.vector.memset(ones_sb[:], 1.0)

            # persistent K^T tiles per (head-pair, block) and V tiles per tk chunk
            kT = [[None] * NB for _ in range(HP)]
            vt = [None] * TCH
            qT_store = [[None] * 4 for _ in range(NB)]

            def qk_group(j, t):
                ps = papool.tile([128, BLK], f32, tag="pa", name="pa")
                for c in range(CCH):
                    nc.tensor.matmul(
                        ps[:],
                        lhsT=wqk_all[:, c * 1024 + t * 128 : c * 1024 + (t + 1) * 128],
                        rhs=xts(j, c),
                        start=(c == 0),
                        stop=(c == CCH - 1),
                    )
                if t < 4:
                    dst = qtpool.tile([128, BLK], bf, tag=f"qT{t}", name=f"qT{t}")
                    qT_store[j][t] = dst
                else:
                    dst = kvpool.tile(
                        [128, BLK], bf, tag=f"kT{t - 4}_{j}", name=f"kT{t - 4}_{j}"
                    )
                    kT[t - 4][j] = dst
                nc.vector.tensor_scalar_add(dst[:], ps[:], bqk_sb[:, t : t + 1])

            def v_group(j, tcl):
                tci = 4 * j + tcl
                ps = papool.tile([128, BLK], f32, tag="pa", name="pa")
                for c in range(CCH):
                    nc.tensor.matmul(
                        ps[:],
                        lhsT=xts(j, c)[:, tcl * 128 : (tcl + 1) * 128],
                        rhs=wv_all[:, c * 512 : (c + 1) * 512],
                        start=(c == 0),
                        stop=(c == CCH - 1),
                    )
                v_ = kvpool.tile([128, H, 65], bf, tag=f"v{tci}", name=f"v{tci}")
                vt[tci] = v_
                nc.vector.tensor_copy(
                    v_[:, :, 64], ones_sb[:, 0:1].to_broadcast([128, H])
                )
                nc.vector.tensor_copy(
                    v_[:, :, :64], ps[:].rearrange("p (h d) -> p h d", d=64)
                )

            def qkv_thunks(j):
                return [lambda t=t: qk_group(j, t) for t in range(8)] + [
                    lambda tcl=tcl: v_group(j, tcl) for tcl in range(4)
                ]

            # prologue: block 0 QKV
            for th in qkv_thunks(0):
                th()

            def proj_thunk(j, oT):
                def _proj():
                    ysb = ypool.tile([128, 8 * BLK], bf, tag="y", name="ysb")
                    for t in range(8):
                        ps = papool.tile([128, BLK], f32, tag="pa", name="pa")
                        for cp in range(4):
                            nc.tensor.matmul(
                                ps[:],
                                lhsT=wpr_all[
                                    :, cp * 1024 + t * 128 : cp * 1024 + (t + 1) * 128
                                ],
                                rhs=oT[cp][:],
                                start=(cp == 0),
                                stop=(cp == 3),
                            )
                        nc.vector.tensor_scalar_add(
                            ysb[:, t * BLK : (t + 1) * BLK], ps[:], bpr_sb[:, t : t + 1]
                        )
                    nc.gpsimd.dma_start(
                        yT[:, j * 8 * BLK : (j + 1) * 8 * BLK], ysb[:]
                    )

                return _proj

            proj_prev = None
            for j in range(NB):
                nchunks = 4 * j + 4
                pending = qkv_thunks(j + 1) if j + 1 < NB else []
                emitted = 0
                total_iters = HP * nchunks
                it = 0

                oT = [
                    otpool.tile([128, BLK], bf, tag=f"oT{hp}", name=f"oT{hp}")
                    for hp in range(HP)
                ]
                qT = qT_store[j]
                for hp in range(HP):
                    if hp == 1 and proj_prev is not None:
                        proj_prev()
                        proj_prev = None
                    o_ps = [
                        opool.tile([65, BLK], f32, tag="o", name="o_ps")
                        for _ in range(2)
                    ]
                    prev = None
                    for tci in range(nchunks):
                        d = tci - 4 * j
                        c0 = 0 if d < 0 else SCORE_C0[d]
                        st = stpool.tile([128, 2 * BLK], f32, tag="st", name="st")
                        for s in range(2):
                            sb = s * BLK
                            kslc = kT[hp][tci // 4][
                                64 * s : 64 * s + 64,
                                (tci % 4) * 128 : (tci % 4 + 1) * 128,
                            ]
                            if d >= 0:
                                # causal triangle: cols [c0, c0+128) of this
                                # chunk get the -1e30 upper-triangular mask
                                nc.tensor.matmul(
                                    st[:, sb + c0 : sb + c0 + 128],
                                    lhsT=ident_sb,
                                    rhs=mask128,
                                    start=True,
                                    stop=False,
                                )
                            nc.tensor.matmul(
                                st[:, sb + c0 : sb + BLK],
                                lhsT=kslc,
                                rhs=qT[hp][64 * s : 64 * s + 64, c0:],
                                start=(d < 0),
                                stop=True,
                                tile_position=(64 * s, 0),
                            )
                        pt = ptpool.tile([128, 2 * BLK], bf, tag="pT", name="pT")
                        if d >= 2:
                            # most of the tile is masked dead; exp only the
                            # live [c0, BLK) span of each head
                            for s in range(2):
                                sb = s * BLK
                                nc.scalar.activation(
                                    pt[:, sb + c0 : sb + BLK],
                                    st[:, sb + c0 : sb + BLK],
                                    Exp,
                                    scale=0.125,
                                )
                        else:
                            nc.scalar.activation(pt[:], st[:], Exp, scale=0.125)
                        if prev is not None:
                            pt_p, tcp, c0p = prev
                            for s in range(2):
                                nc.tensor.matmul(
                                    o_ps[s][:, c0p:],
                                    lhsT=vt[tcp][:, 2 * hp + s, :],
                                    rhs=pt_p[:, s * BLK + c0p : (s + 1) * BLK],
                                    start=(tcp == 0),
                                    stop=(tcp == nchunks - 1),
                                )
                        prev = (pt, tci, c0)
                        # interleave next block's QKV groups into the PE stream
                        it += 1
                        want = (it * len(pending)) // total_iters if pending else 0
                        while emitted < want:
                            pending[emitted]()
                            emitted += 1
                    pt_p, tcp, c0p = prev
                    for s in range(2):
                        nc.tensor.matmul(
                            o_ps[s][:, c0p:],
                            lhsT=vt[tcp][:, 2 * hp + s, :],
                            rhs=pt_p[:, s * BLK + c0p : (s + 1) * BLK],
                            start=(tcp == 0),
                            stop=(tcp == nchunks - 1),
                        )
                    for s in range(2):
                        o_sb = osbpool.tile([65, BLK], f32, tag="osb", name="o_sb")
                        nc.vector.tensor_copy(o_sb[:], o_ps[s][:])
                        rc = rpool.tile([1, BLK], f32, tag="rcp", name="rcp")
                        nc.vector.reciprocal(rc[:], o_sb[64:65, :])
                        bc = rpool.tile([64, BLK], f32, tag="bc", name="bc")
                        nc.gpsimd.partition_broadcast(bc[:], rc[:])
                        nc.vector.tensor_tensor(
                            oT[hp][64 * s : 64 * s + 64, :],
                            o_sb[:64, :],
                            bc[:],
                            Mult,
                        )
                while emitted < len(pending):
                    pending[emitted]()
                    emitted += 1

                # defer this block's output projection into the next block's
                # attention stream (emitted after its first head pair)
                proj_prev = proj_thunk(j, oT)
            proj_prev()

    nc.compile()
    return nc


def _host_inputs(x, W_attn, b_attn, W_proj, b_proj):
    """Build the 8 per-core input maps."""
    x = np.asarray(x, dtype=np.float32)
    W_attn = np.asarray(W_attn, dtype=np.float32)
    b_attn = np.asarray(b_attn, dtype=np.float32)
    W_proj = np.asarray(W_proj, dtype=np.float32)
    b_proj = np.asarray(b_proj, dtype=np.float32)

    p = np.arange(128)[:, None]
    g = np.arange(128)[None, :]
    maskpack = np.zeros((128, 256), dtype=np.float32)
    maskpack[:, 0:128] = np.where(g >= p, 0.0, NEG)
    maskpack[:, 128:256] = np.eye(128, dtype=np.float32)
    maskpack = maskpack.astype(BF16)

    in_maps = []
    for core in range(8):
        b = core // 2
        hh = core % 2
        cs = hh * 512
        wq = W_attn[:, cs : cs + 512]
        wk = W_attn[:, C + cs : C + cs + 512]
        wv_ = W_attn[:, 2 * C + cs : 2 * C + cs + 512]
        bq = b_attn[cs : cs + 512]
        bk = b_attn[C + cs : C + cs + 512]
        bv = b_attn[2 * C + cs : 2 * C + cs + 512]
        wpr_ = W_proj[cs : cs + 512, :]
        bpr_eff = bv @ wpr_ + (b_proj if hh == 0 else 0.0)
        biases = np.concatenate(
            [
                np.concatenate([bq, bk]).reshape(8, 128).T,
                bpr_eff.astype(np.float32).reshape(8, 128).T,
            ],
            axis=1,
        )
        # xT packed: [128, (j, c, f)] with xT_pack[p, j, c, f] = x[b][j*BLK+f, c*128+p]
        xt = x[b].T.reshape(CCH, 128, NB, BLK)          # [c, p, j, f]
        xt_pack = np.ascontiguousarray(xt.transpose(1, 2, 0, 3)).reshape(
            128, NB * CCH * BLK
        )
        # weights packed: [128, (c, n)] with w_pack[p, c, n] = w[c*128+p, n]
        wqk_n = np.concatenate([wq, wk], axis=1)        # [C, 1024]
        wqk_pack = np.ascontiguousarray(
            wqk_n.reshape(CCH, 128, 1024).transpose(1, 0, 2)
        ).reshape(128, CCH * 1024)
        wv_pack = np.ascontiguousarray(
            wv_.reshape(CCH, 128, 512).transpose(1, 0, 2)
        ).reshape(128, CCH * 512)
        wpr_pack = np.ascontiguousarray(
            wpr_.reshape(4, 128, 1024).transpose(1, 0, 2)
        ).reshape(128, 4 * 1024)
        in_maps.append(
            {
                "xT": xt_pack.astype(BF16),
                "wqk": wqk_pack.astype(BF16),
                "wv": wv_pack.astype(BF16),
                "wproj": wpr_pack.astype(BF16),
                "maskpack": maskpack,
                "biases": np.ascontiguousarray(biases.astype(np.float32)),
            }
        )
    return in_maps


def run(x, W_attn, b_attn, W_proj, b_proj, trace=False):
    from concourse.bass_utils import run_bass_kernel_spmd

    if "nc" not in _CACHE:
        _CACHE["nc"] = _build_nc()
    nc = _CACHE["nc"]
    in_maps = _host_inputs(x, W_attn, b_attn, W_proj, b_proj)
    res = run_bass_kernel_spmd(nc, in_maps, core_ids=list(range(8)), trace=False)
    y = np.empty((B, T, C), dtype=np.float32)
    for b in range(B):
        # yT packed per core: [128, (j, t, f)] = y^T[t*128+p, j*BLK+f]
        acc = None
        for hh in range(2):
            yt = res.results[2 * b + hh]["yT"].astype(np.float32)
            yt = yt.reshape(128, NB, 8, BLK).transpose(2, 0, 1, 3).reshape(C, T)
            acc = yt if acc is None else acc + yt
        y[b] = acc.T
    return y, res


def kernel(x, W_attn, b_attn, W_proj, b_proj):
    y, _ = run(x, W_attn, b_attn, W_proj, b_proj, trace=False)
    return y


def make_timed_runner(in_maps=None, nc=None):
    """Build a non-donating jitted SPMD callable with device-resident inputs.

    Returns fn(n) -> wall seconds to execute the kernel n times back-to-back
    (async dispatch, single block at the end). Differential timing
    (wall(n) - wall(1)) / (n - 1) estimates per-execution device time plus
    a per-call dispatch overhead (~400us on this axon setup; calibrate with
    a trivial kernel and subtract).
    """
    import jax
    import numpy as _np
    import concourse.mybir as mybir
    from concourse import bass2jax
    from jax.experimental.shard_map import shard_map
    from jax.sharding import Mesh, PartitionSpec, NamedSharding

    if nc is None:
        if "nc" not in _CACHE:
            _CACHE["nc"] = _build_nc()
        nc = _CACHE["nc"]

    bass2jax.install_neuronx_cc_hook()
    n_cores = 8

    partition_name = nc.partition_id_tensor.name if nc.partition_id_tensor else None
    in_names, out_names, out_avals, zero_outs = [], [], [], []
    for alloc in nc.m.functions[0].allocations:
        if not isinstance(alloc, mybir.MemoryLocationSet):
            continue
        name = alloc.memorylocations[0].name
        if alloc.kind == "ExternalInput":
            if name != partition_name:
                in_names.append(name)
        elif alloc.kind == "ExternalOutput":
            out_names.append(name)
            shape = tuple(alloc.tensor_shape)
            dtype = mybir.dt.np(alloc.dtype)
            out_avals.append(jax.core.ShapedArray(shape, dtype))
            zero_outs.append(_np.zeros(shape, dtype))
    n_params = len(in_names)
    all_names = in_names + out_names
    if partition_name is not None:
        all_names = all_names + [partition_name]

    def _body(*args):
        operands = list(args)
        if partition_name is not None:
            operands.append(bass2jax.partition_id_tensor())
        outs = bass2jax._bass_exec_p.bind(
            *operands,
            out_avals=tuple(out_avals),
            in_names=tuple(all_names),
            out_names=tuple(out_names),
            lowering_input_output_aliases=(),
            sim_require_finite=True,
            sim_require_nnan=True,
            nc=nc,
        )
        return tuple(outs)

    devices = jax.devices()[:n_cores]
    mesh = Mesh(_np.asarray(devices), ("core",))
    spec = PartitionSpec("core")
    sharded = jax.jit(
        shard_map(
            _body,
            mesh=mesh,
            in_specs=(spec,) * (n_params + len(out_names)),
            out_specs=(spec,) * len(out_names),
            check_rep=False,
        ),
        keep_unused=True,
    )
    sh = NamedSharding(mesh, spec)
    dev_args = [
        jax.device_put(
            _np.concatenate([_np.asarray(in_maps[c][nm]) for c in range(n_cores)], 0),
            sh,
        )
        for nm in in_names
    ] + [
        jax.device_put(
            _np.zeros((n_cores * z.shape[0], *z.shape[1:]), z.dtype), sh
        )
        for z in zero_outs
    ]

    import time as _time

    def timed(n):
        out = None
        t0 = _time.perf_counter()
        for _ in range(n):
            out = sharded(*dev_args)
        jax.block_until_ready(out)
        return _time.perf_counter() - t0

    return timed


# revision 23
# speedup vs baseline: 3.4515x; 1.1744x over previous
"""Causal self-attention (B=4, T=2048, C=1024, 16 heads) on 8 trn2 NeuronCores.

Sharding: core i handles batch b=i//2 and head-half hh=i%2 (8 of 16 heads).
Each core computes its 8 heads' attention output projected through its slice
of W_proj rows (a partial sum of y); host adds the two head-half partials per
batch and transposes back.

Layout strategy (matmul operands in bf16, fp32 PSUM accumulation):
  - host pre-transposes x[b] -> xT [C, T]; xT and all weights live in SBUF
    for the whole kernel, loaded by a handful of large DMAs
  - qk^T = W_qk.T @ x (via lhsT=W_qk chunks, rhs=xT chunks): [qk_cols, T]
  - V natural [T, vcols] (via lhsT=xT chunk, rhs=W_v), with a ones column
    per head so the PV matmul also produces the softmax denominator
  - S^T[tk, tq] = K_h @ Q_h^T via lhsT=K^T cols, rhs=Q^T (two heads packed
    into the 128-row PE array with tile_position row groups); both heads of
    a pair share one [128, 1024] PSUM tile so exp runs as a single ACT op
  - causal mask added in PSUM by an identity-weight matmul of a -1e30 mask
  - P^T = exp(S^T/8) on ScalarE (masked entries underflow to exactly 0)
  - O^T[65, tq] accumulates lhsT=V_ext[tk,65], rhs=P^T; row 64 = sum(exp);
    O copies to SBUF immediately (frees the PSUM bank), then reciprocal of
    row 64 on DVE, gpsimd partition_broadcast, multiply on DVE
  - next block's QKV matmul groups are interleaved into the attention
    instruction stream so the in-order PE queue has filler work while
    ScalarE computes exp
  - y^T = W_proj.T @ attn_out^T accumulated over head pairs; bias per
    partition; one output DMA per block
"""

import sys

sys.path.insert(0, "/opt/trn_rl_repo")

import numpy as np
import ml_dtypes

BF16 = ml_dtypes.bfloat16

B, T, C = 4, 2048, 1024
NHEAD_GLOBAL = 16
D = 64
H = 8                    # local heads per core
HP = H // 2              # head pairs
NB = 4                   # tq blocks
BLK = T // NB            # 512
CCH = C // 128           # 8 contraction chunks
TCH = T // 128           # 16 tk chunks
NEG = -1.0e30

SCORE_C0 = [0, 128, 256, 384]     # scores/pv matmul col start per diagonal pos d

_CACHE = {}


def _build_nc():
    import concourse.bass as bass  # noqa: F401
    import concourse.mybir as mybir
    import concourse.tile as tile
    from concourse import bacc

    f32 = mybir.dt.float32
    bf = mybir.dt.bfloat16

    nc = bacc.Bacc("TRN2", target_bir_lowering=False, debug=False)

    xT = nc.dram_tensor("xT", [128, NB * CCH * BLK], bf, kind="ExternalInput").ap()
    wqk = nc.dram_tensor("wqk", [128, CCH * 1024], bf, kind="ExternalInput").ap()
    wv = nc.dram_tensor("wv", [128, CCH * 512], bf, kind="ExternalInput").ap()
    wpr = nc.dram_tensor("wproj", [128, 4 * 1024], bf, kind="ExternalInput").ap()
    mpk = nc.dram_tensor("maskpack", [128, 256], bf, kind="ExternalInput").ap()
    bias = nc.dram_tensor("biases", [128, 16], f32, kind="ExternalInput").ap()
    yT = nc.dram_tensor("yT", [128, NB * 8 * BLK], bf, kind="ExternalOutput").ap()

    Exp = mybir.ActivationFunctionType.Exp
    Mult = mybir.AluOpType.mult

    with tile.TileContext(nc) as tc:
        with (
            tc.tile_pool(name="const", bufs=1) as cpool,
            tc.tile_pool(name="kv", bufs=1) as kvpool,
            tc.tile_pool(name="qt", bufs=2) as qtpool,
            tc.tile_pool(name="pt", bufs=6) as ptpool,
            tc.tile_pool(name="ot", bufs=2) as otpool,
            tc.tile_pool(name="osb", bufs=4) as osbpool,
            tc.tile_pool(name="ysb", bufs=2) as ypool,
            tc.tile_pool(name="rcp", bufs=3) as rpool,
            tc.tile_pool(name="pa_ps", bufs=2, space="PSUM") as papool,
            tc.tile_pool(name="st_ps", bufs=2, space="PSUM") as stpool,
            tc.tile_pool(name="o_ps", bufs=2, space="PSUM") as opool,
        ):
            # ---- resident inputs: flat 2D DMAs in dependency order ----
            xt_all = cpool.tile([128, NB * CCH * BLK], bf, tag="xt", name="xt_all")
            wqk_all = cpool.tile([128, CCH * 1024], bf, tag="wqk", name="wqk_all")
            XS = CCH * BLK

            def xts(jj, c):
                off = (jj * CCH + c) * BLK
                return xt_all[:, off : off + BLK]

            # SP ring: interleave block-0 x chunks with wqk chunks so the
            # first qk matmuls start after ~0.5MB has landed, then stream
            # the remaining x blocks behind compute.
            for c in range(CCH):
                nc.sync.dma_start(xts(0, c), xT[:, c * BLK : (c + 1) * BLK])
                nc.sync.dma_start(
                    wqk_all[:, c * 1024 : (c + 1) * 1024],
                    wqk[:, c * 1024 : (c + 1) * 1024],
                )
            for jj in range(1, NB):
                for ch in range(2):
                    lo = jj * XS + ch * (XS // 2)
                    nc.sync.dma_start(
                        xt_all[:, lo : lo + XS // 2], xT[:, lo : lo + XS // 2]
                    )

            # SWDGE ring: everything else, in need-order
            wv_all = cpool.tile([128, CCH * 512], bf, tag="wv", name="wv_all")
            nc.gpsimd.dma_start(wv_all[:, : 4 * 512], wv[:, : 4 * 512])
            nc.gpsimd.dma_start(wv_all[:, 4 * 512 :], wv[:, 4 * 512 :])
            mp_sb = cpool.tile([128, 256], bf, tag="mpk", name="mp_sb")
            nc.gpsimd.dma_start(mp_sb[:], mpk[:, :])
            mask128 = mp_sb[:, 0:128]
            ident_sb = mp_sb[:, 128:256]
            bias_sb = cpool.tile([128, 16], f32, tag="bias", name="bias_sb")
            nc.gpsimd.dma_start(bias_sb[:], bias[:, :])
            wpr_all = cpool.tile([128, 4 * 1024], bf, tag="wpr", name="wpr_all")
            nc.gpsimd.dma_start(wpr_all[:, : 2 * 1024], wpr[:, : 2 * 1024])
            nc.gpsimd.dma_start(wpr_all[:, 2 * 1024 :], wpr[:, 2 * 1024 :])
            bqk_sb = bias_sb[:, 0:8]
            bpr_sb = bias_sb[:, 8:16]

            ones_sb = cpool.tile([128, 1], f32, tag="ones", name="ones")
            nc.vector.memset(ones_sb[:], 1.0)

            # persistent K^T tiles per (head-pair, block) and V tiles per tk chunk
            kT = [[None] * NB for _ in range(HP)]
            vt = [None] * TCH
            qT_store = [[None] * 4 for _ in range(NB)]

            def qk_group(j, t):
                ps = papool.tile([128, BLK], f32, tag="pa", name="pa")
                for c in range(CCH):
                    nc.tensor.matmul(
                        ps[:],
                        lhsT=wqk_all[:, c * 1024 + t * 128 : c * 1024 + (t + 1) * 128],
                        rhs=xts(j, c),
                        start=(c == 0),
                        stop=(c == CCH - 1),
                    )
                if t < 4:
                    dst = qtpool.tile([128, BLK], bf, tag=f"qT{t}", name=f"qT{t}")
                    qT_store[j][t] = dst
                else:
                    dst = kvpool.tile(
                        [128, BLK], bf, tag=f"kT{t - 4}_{j}", name=f"kT{t - 4}_{j}"
                    )
                    kT[t - 4][j] = dst
                nc.vector.tensor_scalar_add(dst[:], ps[:], bqk_sb[:, t : t + 1])

            def v_group(j, tcl):
                tci = 4 * j + tcl
                ps = papool.tile([128, BLK], f32, tag="pa", name="pa")
                for c in range(CCH):
                    nc.tensor.matmul(
                        ps[:],
                        lhsT=xts(j, c)[:, tcl * 128 : (tcl + 1) * 128],
                        rhs=wv_all[:, c * 512 : (c + 1) * 512],
                        start=(c == 0),
                        stop=(c == CCH - 1),
                    )
                v_ = kvpool.tile([128, H, 65], bf, tag=f"v{tci}", name=f"v{tci}")
                vt[tci] = v_
                nc.vector.tensor_copy(
                    v_[:, :, 64], ones_sb[:, 0:1].to_broadcast([128, H])
                )
                nc.vector.tensor_copy(
                    v_[:, :, :64], ps[:].rearrange("p (h d) -> p h d", d=64)
                )

            def qkv_thunks(j):
                # attention pair 0 needs the k tiles (t=4..7), qT0 (t=0) and
                # V first; qT1..3 are only needed by later pairs
                order = [4, 5, 6, 7, 0]
                thunks = [lambda t=t: qk_group(j, t) for t in order]
                thunks += [lambda tcl=tcl: v_group(j, tcl) for tcl in range(4)]
                thunks += [lambda t=t: qk_group(j, t) for t in (1, 2, 3)]
                return thunks

            # prologue: block 0 QKV
            for th in qkv_thunks(0):
                th()

            def proj_thunks(j, oT):
                """8 projection-group thunks for block j; each emits its own
                output DMA so the write streams out tile by tile."""
                cell = {}

                def group(t):
                    if "ysb" not in cell:
                        cell["ysb"] = ypool.tile(
                            [128, 8 * BLK], bf, tag="y", name="ysb"
                        )
                    ysb = cell["ysb"]
                    ps = papool.tile([128, BLK], f32, tag="pa", name="pa")
                    for cp in range(4):
                        nc.tensor.matmul(
                            ps[:],
                            lhsT=wpr_all[
                                :, cp * 1024 + t * 128 : cp * 1024 + (t + 1) * 128
                            ],
                            rhs=oT[cp][:],
                            start=(cp == 0),
                            stop=(cp == 3),
                        )
                    nc.vector.tensor_scalar_add(
                        ysb[:, t * BLK : (t + 1) * BLK], ps[:], bpr_sb[:, t : t + 1]
                    )
                    nc.gpsimd.dma_start(
                        yT[:, (j * 8 + t) * BLK : (j * 8 + t + 1) * BLK],
                        ysb[:, t * BLK : (t + 1) * BLK],
                    )

                return [lambda t=t: group(t) for t in range(8)]

            proj_prev = []
            for j in range(NB):
                nchunks = 4 * j + 4
                pending = list(proj_prev)
                proj_prev = []
                if j + 1 < NB:
                    pending.extend(qkv_thunks(j + 1))
                emitted = 0
                total_iters = HP * nchunks
                it = 0

                oT = [
                    otpool.tile([128, BLK], bf, tag=f"oT{hp}", name=f"oT{hp}")
                    for hp in range(HP)
                ]
                qT = qT_store[j]
                for hp in range(HP):
                    o_ps = [
                        opool.tile([65, BLK], f32, tag="o", name="o_ps")
                        for _ in range(2)
                    ]
                    prev = None
                    for tci in range(nchunks):
                        d = tci - 4 * j
                        c0 = 0 if d < 0 else SCORE_C0[d]
                        st = stpool.tile([128, 2 * BLK], f32, tag="st", name="st")
                        for s in range(2):
                            sb = s * BLK
                            kslc = kT[hp][tci // 4][
                                64 * s : 64 * s + 64,
                                (tci % 4) * 128 : (tci % 4 + 1) * 128,
                            ]
                            if d >= 0:
                                # causal triangle: cols [c0, c0+128) of this
                                # chunk get the -1e30 upper-triangular mask
                                nc.tensor.matmul(
                                    st[:, sb + c0 : sb + c0 + 128],
                                    lhsT=ident_sb,
                                    rhs=mask128,
                                    start=True,
                                    stop=False,
                                )
                            nc.tensor.matmul(
                                st[:, sb + c0 : sb + BLK],
                                lhsT=kslc,
                                rhs=qT[hp][64 * s : 64 * s + 64, c0:],
                                start=(d < 0),
                                stop=True,
                                tile_position=(64 * s, 0),
                            )
                        pt = ptpool.tile([128, 2 * BLK], bf, tag="pT", name="pT")
                        if d >= 2:
                            # most of the tile is masked dead; exp only the
                            # live [c0, BLK) span of each head
                            for s in range(2):
                                sb = s * BLK
                                nc.scalar.activation(
                                    pt[:, sb + c0 : sb + BLK],
                                    st[:, sb + c0 : sb + BLK],
                                    Exp,
                                    scale=0.125,
                                )
                        else:
                            nc.scalar.activation(pt[:], st[:], Exp, scale=0.125)
                        if prev is not None:
                            pt_p, tcp, c0p = prev
                            for s in range(2):
                                nc.tensor.matmul(
                                    o_ps[s][:, c0p:],
                                    lhsT=vt[tcp][:, 2 * hp + s, :],
                                    rhs=pt_p[:, s * BLK + c0p : (s + 1) * BLK],
                                    start=(tcp == 0),
                                    stop=(tcp == nchunks - 1),
                                )
                        prev = (pt, tci, c0)
                        # interleave next block's QKV groups into the PE stream
                        it += 1
                        want = (it * len(pending)) // total_iters if pending else 0
                        while emitted < want:
                            pending[emitted]()
                            emitted += 1
                    pt_p, tcp, c0p = prev
                    for s in range(2):
                        nc.tensor.matmul(
                            o_ps[s][:, c0p:],
                            lhsT=vt[tcp][:, 2 * hp + s, :],
                            rhs=pt_p[:, s * BLK + c0p : (s + 1) * BLK],
                            start=(tcp == 0),
                            stop=(tcp == nchunks - 1),
                        )
                    for s in range(2):
                        o_sb = osbpool.tile([65, BLK], f32, tag="osb", name="o_sb")
                        nc.vector.tensor_copy(o_sb[:], o_ps[s][:])
                        rc = rpool.tile([1, BLK], f32, tag="rcp", name="rcp")
                        nc.vector.reciprocal(rc[:], o_sb[64:65, :])
                        bc = rpool.tile([64, BLK], f32, tag="bc", name="bc")
                        nc.gpsimd.partition_broadcast(bc[:], rc[:])
                        nc.vector.tensor_tensor(
                            oT[hp][64 * s : 64 * s + 64, :],
                            o_sb[:64, :],
                            bc[:],
                            Mult,
                        )
                while emitted < len(pending):
                    pending[emitted]()
                    emitted += 1

                # defer this block's output projection into the next block's
                # attention stream (spread as filler groups)
                proj_prev = proj_thunks(j, oT)
            for th in proj_prev:
                th()

    nc.compile()
    return nc


def _host_inputs(x, W_attn, b_attn, W_proj, b_proj):
    """Build the 8 per-core input maps."""
    x = np.asarray(x, dtype=np.float32)
    W_attn = np.asarray(W_attn, dtype=np.float32)
    b_attn = np.asarray(b_attn, dtype=np.float32)
    W_proj = np.asarray(W_proj, dtype=np.float32)
    b_proj = np.asarray(b_proj, dtype=np.float32)

    p = np.arange(128)[:, None]
    g = np.arange(128)[None, :]
    maskpack = np.zeros((128, 256), dtype=np.float32)
    maskpack[:, 0:128] = np.where(g >= p, 0.0, NEG)
    maskpack[:, 128:256] = np.eye(128, dtype=np.float32)
    maskpack = maskpack.astype(BF16)

    in_maps = []
    for core in range(8):
        b = core // 2
        hh = core % 2
        cs = hh * 512
        wq = W_attn[:, cs : cs + 512]
        wk = W_attn[:, C + cs : C + cs + 512]
        wv_ = W_attn[:, 2 * C + cs : 2 * C + cs + 512]
        bq = b_attn[cs : cs + 512]
        bk = b_attn[C + cs : C + cs + 512]
        bv = b_attn[2 * C + cs : 2 * C + cs + 512]
        wpr_ = W_proj[cs : cs + 512, :]
        bpr_eff = bv @ wpr_ + (b_proj if hh == 0 else 0.0)
        biases = np.concatenate(
            [
                np.concatenate([bq, bk]).reshape(8, 128).T,
                bpr_eff.astype(np.float32).reshape(8, 128).T,
            ],
            axis=1,
        )
        # xT packed: [128, (j, c, f)] with xT_pack[p, j, c, f] = x[b][j*BLK+f, c*128+p]
        xt = x[b].T.reshape(CCH, 128, NB, BLK)          # [c, p, j, f]
        xt_pack = np.ascontiguousarray(xt.transpose(1, 2, 0, 3)).reshape(
            128, NB * CCH * BLK
        )
        # weights packed: [128, (c, n)] with w_pack[p, c, n] = w[c*128+p, n]
        wqk_n = np.concatenate([wq, wk], axis=1)        # [C, 1024]
        wqk_pack = np.ascontiguousarray(
            wqk_n.reshape(CCH, 128, 1024).transpose(1, 0, 2)
        ).reshape(128, CCH * 1024)
        wv_pack = np.ascontiguousarray(
            wv_.reshape(CCH, 128, 512).transpose(1, 0, 2)
        ).reshape(128, CCH * 512)
        wpr_pack = np.ascontiguousarray(
            wpr_.reshape(4, 128, 1024).transpose(1, 0, 2)
        ).reshape(128, 4 * 1024)
        in_maps.append(
            {
                "xT": xt_pack.astype(BF16),
                "wqk": wqk_pack.astype(BF16),
                "wv": wv_pack.astype(BF16),
                "wproj": wpr_pack.astype(BF16),
                "maskpack": maskpack,
                "biases": np.ascontiguousarray(biases.astype(np.float32)),
            }
        )
    return in_maps


def run(x, W_attn, b_attn, W_proj, b_proj, trace=False):
    from concourse.bass_utils import run_bass_kernel_spmd

    if "nc" not in _CACHE:
        _CACHE["nc"] = _build_nc()
    nc = _CACHE["nc"]
    in_maps = _host_inputs(x, W_attn, b_attn, W_proj, b_proj)
    res = run_bass_kernel_spmd(nc, in_maps, core_ids=list(range(8)), trace=False)
    y = np.empty((B, T, C), dtype=np.float32)
    for b in range(B):
        # yT packed per core: [128, (j, t, f)] = y^T[t*128+p, j*BLK+f]
        acc = None
        for hh in range(2):
            yt = res.results[2 * b + hh]["yT"].astype(np.float32)
            yt = yt.reshape(128, NB, 8, BLK).transpose(2, 0, 1, 3).reshape(C, T)
            acc = yt if acc is None else acc + yt
        y[b] = acc.T
    return y, res


def kernel(x, W_attn, b_attn, W_proj, b_proj):
    y, _ = run(x, W_attn, b_attn, W_proj, b_proj, trace=False)
    return y


def make_timed_runner(in_maps=None, nc=None):
    """Build a non-donating jitted SPMD callable with device-resident inputs.

    Returns fn(n) -> wall seconds to execute the kernel n times back-to-back
    (async dispatch, single block at the end). Differential timing
    (wall(n) - wall(1)) / (n - 1) estimates per-execution device time plus
    a per-call dispatch overhead (~400us on this axon setup; calibrate with
    a trivial kernel and subtract).
    """
    import jax
    import numpy as _np
    import concourse.mybir as mybir
    from concourse import bass2jax
    from jax.experimental.shard_map import shard_map
    from jax.sharding import Mesh, PartitionSpec, NamedSharding

    if nc is None:
        if "nc" not in _CACHE:
            _CACHE["nc"] = _build_nc()
        nc = _CACHE["nc"]

    bass2jax.install_neuronx_cc_hook()
    n_cores = 8

    partition_name = nc.partition_id_tensor.name if nc.partition_id_tensor else None
    in_names, out_names, out_avals, zero_outs = [], [], [], []
    for alloc in nc.m.functions[0].allocations:
        if not isinstance(alloc, mybir.MemoryLocationSet):
            continue
        name = alloc.memorylocations[0].name
        if alloc.kind == "ExternalInput":
            if name != partition_name:
                in_names.append(name)
        elif alloc.kind == "ExternalOutput":
            out_names.append(name)
            shape = tuple(alloc.tensor_shape)
            dtype = mybir.dt.np(alloc.dtype)
            out_avals.append(jax.core.ShapedArray(shape, dtype))
            zero_outs.append(_np.zeros(shape, dtype))
    n_params = len(in_names)
    all_names = in_names + out_names
    if partition_name is not None:
        all_names = all_names + [partition_name]

    def _body(*args):
        operands = list(args)
        if partition_name is not None:
            operands.append(bass2jax.partition_id_tensor())
        outs = bass2jax._bass_exec_p.bind(
            *operands,
            out_avals=tuple(out_avals),
            in_names=tuple(all_names),
            out_names=tuple(out_names),
            lowering_input_output_aliases=(),
            sim_require_finite=True,
            sim_require_nnan=True,
            nc=nc,
        )
        return tuple(outs)

    devices = jax.devices()[:n_cores]
    mesh = Mesh(_np.asarray(devices), ("core",))
    spec = PartitionSpec("core")
    sharded = jax.jit(
        shard_map(
            _body,
            mesh=mesh,
            in_specs=(spec,) * (n_params + len(out_names)),
            out_specs=(spec,) * len(out_names),
            check_rep=False,
        ),
        keep_unused=True,
    )
    sh = NamedSharding(mesh, spec)
    dev_args = [
        jax.device_put(
            _np.concatenate([_np.asarray(in_maps[c][nm]) for c in range(n_cores)], 0),
            sh,
        )
        for nm in in_names
    ] + [
        jax.device_put(
            _np.zeros((n_cores * z.shape[0], *z.shape[1:]), z.dtype), sh
        )
        for z in zero_outs
    ]

    import time as _time

    def timed(n):
        out = None
        t0 = _time.perf_counter()
        for _ in range(n):
            out = sharded(*dev_args)
        jax.block_until_ready(out)
        return _time.perf_counter() - t0

    return timed


# revision 25
# speedup vs baseline: 4.1639x; 1.2064x over previous
"""Causal self-attention (B=4, T=2048, C=1024, 16 heads) on 8 trn2 NeuronCores.

Sharding: core i handles batch b=i//2 and head-half hh=i%2 (8 of 16 heads).
Each core computes its 8 heads' attention output projected through its slice
of W_proj rows (a partial sum of y); host adds the two head-half partials per
batch and transposes back.

Layout strategy (matmul operands in bf16, fp32 PSUM accumulation):
  - host pre-transposes x[b] -> xT [C, T]; xT and all weights live in SBUF
    for the whole kernel, loaded by a handful of large DMAs
  - qk^T = W_qk.T @ x (via lhsT=W_qk chunks, rhs=xT chunks): [qk_cols, T]
  - V natural [T, vcols] (via lhsT=xT chunk, rhs=W_v), with a ones column
    per head so the PV matmul also produces the softmax denominator
  - S^T[tk, tq] = K_h @ Q_h^T via lhsT=K^T cols, rhs=Q^T (two heads packed
    into the 128-row PE array with tile_position row groups); both heads of
    a pair share one [128, 1024] PSUM tile so exp runs as a single ACT op
  - causal mask added in PSUM by an identity-weight matmul of a -1e30 mask
  - P^T = exp(S^T/8) on ScalarE (masked entries underflow to exactly 0)
  - O^T[65, tq] accumulates lhsT=V_ext[tk,65], rhs=P^T; row 64 = sum(exp);
    O copies to SBUF immediately (frees the PSUM bank), then reciprocal of
    row 64 on DVE, gpsimd partition_broadcast, multiply on DVE
  - next block's QKV matmul groups are interleaved into the attention
    instruction stream so the in-order PE queue has filler work while
    ScalarE computes exp
  - y^T = W_proj.T @ attn_out^T accumulated over head pairs; bias per
    partition; one output DMA per block
"""

import sys

sys.path.insert(0, "/opt/trn_rl_repo")

import numpy as np
import ml_dtypes

BF16 = ml_dtypes.bfloat16

B, T, C = 4, 2048, 1024
NHEAD_GLOBAL = 16
D = 64
H = 8                    # local heads per core
HP = H // 2              # head pairs
NB = 4                   # tq blocks
BLK = T // NB            # 512
CCH = C // 128           # 8 contraction chunks
TCH = T // 128           # 16 tk chunks
NEG = -1.0e30

SCORE_C0 = [0, 128, 256, 384]     # scores/pv matmul col start per diagonal pos d

_CACHE = {}


def _build_nc():
    import concourse.bass as bass  # noqa: F401
    import concourse.mybir as mybir
    import concourse.tile as tile
    from concourse import bacc

    f32 = mybir.dt.float32
    bf = mybir.dt.bfloat16

    nc = bacc.Bacc("TRN2", target_bir_lowering=False, debug=False)

    xT = nc.dram_tensor("xT", [128, NB * CCH * BLK], bf, kind="ExternalInput").ap()
    wqk = nc.dram_tensor("wqk", [128, CCH * 1024], bf, kind="ExternalInput").ap()
    wv = nc.dram_tensor("wv", [128, CCH * 512], bf, kind="ExternalInput").ap()
    wpr = nc.dram_tensor("wproj", [128, 4 * 1024], bf, kind="ExternalInput").ap()
    mpk = nc.dram_tensor("maskpack", [128, 256], bf, kind="ExternalInput").ap()
    bias = nc.dram_tensor("biases", [128, 16], f32, kind="ExternalInput").ap()
    yT = nc.dram_tensor("yT", [128, NB * 8 * BLK], bf, kind="ExternalOutput").ap()

    Exp = mybir.ActivationFunctionType.Exp
    Mult = mybir.AluOpType.mult

    with tile.TileContext(nc) as tc:
        with (
            tc.tile_pool(name="const", bufs=1) as cpool,
            tc.tile_pool(name="kv", bufs=1) as kvpool,
            tc.tile_pool(name="qt", bufs=2) as qtpool,
            tc.tile_pool(name="pt", bufs=8) as ptpool,
            tc.tile_pool(name="ot", bufs=2) as otpool,
            tc.tile_pool(name="osb", bufs=6) as osbpool,
            tc.tile_pool(name="ysb", bufs=3) as ypool,
            tc.tile_pool(name="rcp", bufs=4) as rpool,
            tc.tile_pool(name="pa_ps", bufs=2, space="PSUM") as papool,
            tc.tile_pool(name="st_ps", bufs=2, space="PSUM") as stpool,
            tc.tile_pool(name="o_ps", bufs=2, space="PSUM") as opool,
        ):
            # ---- resident inputs: flat 2D DMAs in dependency order ----
            xt_all = cpool.tile([128, NB * CCH * BLK], bf, tag="xt", name="xt_all")
            wqk_all = cpool.tile([128, CCH * 1024], bf, tag="wqk", name="wqk_all")
            XS = CCH * BLK

            def xts(jj, c):
                off = (jj * CCH + c) * BLK
                return xt_all[:, off : off + BLK]

            # SP ring: interleave block-0 x chunks with wqk chunks so the
            # first qk matmuls start after ~0.5MB has landed, then stream
            # the remaining x blocks behind compute.
            for c in range(CCH):
                nc.sync.dma_start(xts(0, c), xT[:, c * BLK : (c + 1) * BLK])
                nc.sync.dma_start(
                    wqk_all[:, c * 1024 : (c + 1) * 1024],
                    wqk[:, c * 1024 : (c + 1) * 1024],
                )
            for jj in range(1, NB):
                for ch in range(2):
                    lo = jj * XS + ch * (XS // 2)
                    nc.sync.dma_start(
                        xt_all[:, lo : lo + XS // 2], xT[:, lo : lo + XS // 2]
                    )

            # SWDGE ring: everything else, in need-order
            wv_all = cpool.tile([128, CCH * 512], bf, tag="wv", name="wv_all")
            nc.gpsimd.dma_start(wv_all[:, : 4 * 512], wv[:, : 4 * 512])
            nc.gpsimd.dma_start(wv_all[:, 4 * 512 :], wv[:, 4 * 512 :])
            mp_sb = cpool.tile([128, 256], bf, tag="mpk", name="mp_sb")
            nc.gpsimd.dma_start(mp_sb[:], mpk[:, :])
            mask128 = mp_sb[:, 0:128]
            ident_sb = mp_sb[:, 128:256]
            bias_sb = cpool.tile([128, 16], f32, tag="bias", name="bias_sb")
            nc.gpsimd.dma_start(bias_sb[:], bias[:, :])
            wpr_all = cpool.tile([128, 4 * 1024], bf, tag="wpr", name="wpr_all")
            nc.gpsimd.dma_start(wpr_all[:, : 2 * 1024], wpr[:, : 2 * 1024])
            nc.gpsimd.dma_start(wpr_all[:, 2 * 1024 :], wpr[:, 2 * 1024 :])
            bqk_sb = bias_sb[:, 0:8]
            bpr_sb = bias_sb[:, 8:16]

            ones_sb = cpool.tile([128, 1], f32, tag="ones", name="ones")
            nc.vector.memset(ones_sb[:], 1.0)

            # persistent K^T tiles per (head-pair, block) and V tiles per tk chunk
            kT = [[None] * NB for _ in range(HP)]
            vt = [None] * TCH
            qT_store = [[None] * 4 for _ in range(NB)]

            def qk_group(j, t):
                ps = papool.tile([128, BLK], f32, tag="pa", name="pa")
                for c in range(CCH):
                    nc.tensor.matmul(
                        ps[:],
                        lhsT=wqk_all[:, c * 1024 + t * 128 : c * 1024 + (t + 1) * 128],
                        rhs=xts(j, c),
                        start=(c == 0),
                        stop=(c == CCH - 1),
                    )
                if t < 4:
                    dst = qtpool.tile([128, BLK], bf, tag=f"qT{t}", name=f"qT{t}")
                    qT_store[j][t] = dst
                else:
                    dst = kvpool.tile(
                        [128, BLK], bf, tag=f"kT{t - 4}_{j}", name=f"kT{t - 4}_{j}"
                    )
                    kT[t - 4][j] = dst
                nc.vector.tensor_scalar_add(dst[:], ps[:], bqk_sb[:, t : t + 1])

            def v_group(j, tcl):
                tci = 4 * j + tcl
                ps = papool.tile([128, BLK], f32, tag="pa", name="pa")
                for c in range(CCH):
                    nc.tensor.matmul(
                        ps[:],
                        lhsT=xts(j, c)[:, tcl * 128 : (tcl + 1) * 128],
                        rhs=wv_all[:, c * 512 : (c + 1) * 512],
                        start=(c == 0),
                        stop=(c == CCH - 1),
                    )
                v_ = kvpool.tile([128, H, 65], bf, tag=f"v{tci}", name=f"v{tci}")
                vt[tci] = v_
                nc.vector.tensor_copy(
                    v_[:, :, 64], ones_sb[:, 0:1].to_broadcast([128, H])
                )
                nc.vector.tensor_copy(
                    v_[:, :, :64], ps[:].rearrange("p (h d) -> p h d", d=64)
                )

            def qkv_thunks(j):
                # attention pair 0 needs the k tiles (t=4..7), qT0 (t=0) and
                # V first; qT1..3 are only needed by later pairs
                order = [4, 5, 6, 7, 0]
                thunks = [lambda t=t: qk_group(j, t) for t in order]
                thunks += [lambda tcl=tcl: v_group(j, tcl) for tcl in range(4)]
                thunks += [lambda t=t: qk_group(j, t) for t in (1, 2, 3)]
                return thunks

            # prologue: block 0 QKV
            for th in qkv_thunks(0):
                th()

            def proj_thunks(j, oT):
                """8 projection-group thunks for block j; each emits its own
                output DMA so the write streams out tile by tile."""
                cell = {}

                def group(t):
                    if "ysb" not in cell:
                        cell["ysb"] = ypool.tile(
                            [128, 8 * BLK], bf, tag="y", name="ysb"
                        )
                    ysb = cell["ysb"]
                    ps = papool.tile([128, BLK], f32, tag="pa", name="pa")
                    for cp in range(4):
                        nc.tensor.matmul(
                            ps[:],
                            lhsT=wpr_all[
                                :, cp * 1024 + t * 128 : cp * 1024 + (t + 1) * 128
                            ],
                            rhs=oT[cp][:],
                            start=(cp == 0),
                            stop=(cp == 3),
                        )
                    nc.vector.tensor_scalar_add(
                        ysb[:, t * BLK : (t + 1) * BLK], ps[:], bpr_sb[:, t : t + 1]
                    )
                    nc.gpsimd.dma_start(
                        yT[:, (j * 8 + t) * BLK : (j * 8 + t + 1) * BLK],
                        ysb[:, t * BLK : (t + 1) * BLK],
                    )

                return [lambda t=t: group(t) for t in range(8)]

            proj_prev = []
            for j in range(NB):
                nchunks = 4 * j + 4
                pending = list(proj_prev)
                proj_prev = []
                if j + 1 < NB:
                    pending.extend(qkv_thunks(j + 1))
                emitted = 0
                total_iters = HP * nchunks
                it = 0
                # block 0: its fillers need the x slab of block 1, which is
                # still streaming in — delay them to the back half so a slow
                # DMA cannot stall the in-order PE queue mid-attention
                start_at = total_iters // 2 if j == 0 else 0

                oT = [
                    otpool.tile([128, BLK], bf, tag=f"oT{hp}", name=f"oT{hp}")
                    for hp in range(HP)
                ]
                qT = qT_store[j]
                for hp in range(HP):
                    o_ps = [
                        opool.tile([65, BLK], f32, tag="o", name="o_ps")
                        for _ in range(2)
                    ]
                    prev = None
                    for tci in range(nchunks):
                        d = tci - 4 * j
                        c0 = 0 if d < 0 else SCORE_C0[d]
                        st = stpool.tile([128, 2 * BLK], f32, tag="st", name="st")
                        for s in range(2):
                            sb = s * BLK
                            kslc = kT[hp][tci // 4][
                                64 * s : 64 * s + 64,
                                (tci % 4) * 128 : (tci % 4 + 1) * 128,
                            ]
                            if d >= 0:
                                # causal triangle: cols [c0, c0+128) of this
                                # chunk get the -1e30 upper-triangular mask
                                nc.tensor.matmul(
                                    st[:, sb + c0 : sb + c0 + 128],
                                    lhsT=ident_sb,
                                    rhs=mask128,
                                    start=True,
                                    stop=False,
                                )
                            nc.tensor.matmul(
                                st[:, sb + c0 : sb + BLK],
                                lhsT=kslc,
                                rhs=qT[hp][64 * s : 64 * s + 64, c0:],
                                start=(d < 0),
                                stop=True,
                                tile_position=(64 * s, 0),
                            )
                        pt = ptpool.tile([128, 2 * BLK], bf, tag="pT", name="pT")
                        if d >= 2:
                            # most of the tile is masked dead; exp only the
                            # live [c0, BLK) span of each head
                            for s in range(2):
                                sb = s * BLK
                                nc.scalar.activation(
                                    pt[:, sb + c0 : sb + BLK],
                                    st[:, sb + c0 : sb + BLK],
                                    Exp,
                                    scale=0.125,
                                )
                        else:
                            nc.scalar.activation(pt[:], st[:], Exp, scale=0.125)
                        if prev is not None:
                            pt_p, tcp, c0p = prev
                            for s in range(2):
                                nc.tensor.matmul(
                                    o_ps[s][:, c0p:],
                                    lhsT=vt[tcp][:, 2 * hp + s, :],
                                    rhs=pt_p[:, s * BLK + c0p : (s + 1) * BLK],
                                    start=(tcp == 0),
                                    stop=(tcp == nchunks - 1),
                                )
                        prev = (pt, tci, c0)
                        # interleave next block's QKV groups into the PE stream
                        it += 1
                        want = (
                            (max(0, it - start_at) * len(pending))
                            // (total_iters - start_at)
                            if pending
                            else 0
                        )
                        while emitted < want:
                            pending[emitted]()
                            emitted += 1
                    pt_p, tcp, c0p = prev
                    for s in range(2):
                        nc.tensor.matmul(
                            o_ps[s][:, c0p:],
                            lhsT=vt[tcp][:, 2 * hp + s, :],
                            rhs=pt_p[:, s * BLK + c0p : (s + 1) * BLK],
                            start=(tcp == 0),
                            stop=(tcp == nchunks - 1),
                        )
                    for s in range(2):
                        o_sb = osbpool.tile([65, BLK], f32, tag="osb", name="o_sb")
                        nc.vector.tensor_copy(o_sb[:], o_ps[s][:])
                        rc = rpool.tile([1, BLK], f32, tag="rcp", name="rcp")
                        nc.vector.reciprocal(rc[:], o_sb[64:65, :])
                        bc = rpool.tile([64, BLK], f32, tag="bc", name="bc")
                        nc.gpsimd.partition_broadcast(bc[:], rc[:])
                        nc.vector.tensor_tensor(
                            oT[hp][64 * s : 64 * s + 64, :],
                            o_sb[:64, :],
                            bc[:],
                            Mult,
                        )
                while emitted < len(pending):
                    pending[emitted]()
                    emitted += 1

                # defer this block's output projection into the next block's
                # attention stream (spread as filler groups)
                proj_prev = proj_thunks(j, oT)
            for th in proj_prev:
                th()

    nc.compile()
    return nc


def _host_inputs(x, W_attn, b_attn, W_proj, b_proj):
    """Build the 8 per-core input maps."""
    x = np.asarray(x, dtype=np.float32)
    W_attn = np.asarray(W_attn, dtype=np.float32)
    b_attn = np.asarray(b_attn, dtype=np.float32)
    W_proj = np.asarray(W_proj, dtype=np.float32)
    b_proj = np.asarray(b_proj, dtype=np.float32)

    p = np.arange(128)[:, None]
    g = np.arange(128)[None, :]
    maskpack = np.zeros((128, 256), dtype=np.float32)
    maskpack[:, 0:128] = np.where(g >= p, 0.0, NEG)
    maskpack[:, 128:256] = np.eye(128, dtype=np.float32)
    maskpack = maskpack.astype(BF16)

    in_maps = []
    for core in range(8):
        b = core // 2
        hh = core % 2
        cs = hh * 512
        wq = W_attn[:, cs : cs + 512]
        wk = W_attn[:, C + cs : C + cs + 512]
        wv_ = W_attn[:, 2 * C + cs : 2 * C + cs + 512]
        bq = b_attn[cs : cs + 512]
        bk = b_attn[C + cs : C + cs + 512]
        bv = b_attn[2 * C + cs : 2 * C + cs + 512]
        wpr_ = W_proj[cs : cs + 512, :]
        bpr_eff = bv @ wpr_ + (b_proj if hh == 0 else 0.0)
        biases = np.concatenate(
            [
                np.concatenate([bq, bk]).reshape(8, 128).T,
                bpr_eff.astype(np.float32).reshape(8, 128).T,
            ],
            axis=1,
        )
        # xT packed: [128, (j, c, f)] with xT_pack[p, j, c, f] = x[b][j*BLK+f, c*128+p]
        xt = x[b].T.reshape(CCH, 128, NB, BLK)          # [c, p, j, f]
        xt_pack = np.ascontiguousarray(xt.transpose(1, 2, 0, 3)).reshape(
            128, NB * CCH * BLK
        )
        # weights packed: [128, (c, n)] with w_pack[p, c, n] = w[c*128+p, n]
        wqk_n = np.concatenate([wq, wk], axis=1)        # [C, 1024]
        wqk_pack = np.ascontiguousarray(
            wqk_n.reshape(CCH, 128, 1024).transpose(1, 0, 2)
        ).reshape(128, CCH * 1024)
        wv_pack = np.ascontiguousarray(
            wv_.reshape(CCH, 128, 512).transpose(1, 0, 2)
        ).reshape(128, CCH * 512)
        wpr_pack = np.ascontiguousarray(
            wpr_.reshape(4, 128, 1024).transpose(1, 0, 2)
        ).reshape(128, 4 * 1024)
        in_maps.append(
            {
                "xT": xt_pack.astype(BF16),
                "wqk": wqk_pack.astype(BF16),
                "wv": wv_pack.astype(BF16),
                "wproj": wpr_pack.astype(BF16),
                "maskpack": maskpack,
                "biases": np.ascontiguousarray(biases.astype(np.float32)),
            }
        )
    return in_maps


def run(x, W_attn, b_attn, W_proj, b_proj, trace=False):
    from concourse.bass_utils import run_bass_kernel_spmd

    if "nc" not in _CACHE:
        _CACHE["nc"] = _build_nc()
    nc = _CACHE["nc"]
    in_maps = _host_inputs(x, W_attn, b_attn, W_proj, b_proj)
    res = run_bass_kernel_spmd(nc, in_maps, core_ids=list(range(8)), trace=False)
    y = np.empty((B, T, C), dtype=np.float32)
    for b in range(B):
        # yT packed per core: [128, (j, t, f)] = y^T[t*128+p, j*BLK+f]
        acc = None
        for hh in range(2):
            yt = res.results[2 * b + hh]["yT"].astype(np.float32)
            yt = yt.reshape(128, NB, 8, BLK).transpose(2, 0, 1, 3).reshape(C, T)
            acc = yt if acc is None else acc + yt
        y[b] = acc.T
    return y, res


def kernel(x, W_attn, b_attn, W_proj, b_proj):
    y, _ = run(x, W_attn, b_attn, W_proj, b_proj, trace=False)
    return y


def make_timed_runner(in_maps=None, nc=None):
    """Build a non-donating jitted SPMD callable with device-resident inputs.

    Returns fn(n) -> wall seconds to execute the kernel n times back-to-back
    (async dispatch, single block at the end). Differential timing
    (wall(n) - wall(1)) / (n - 1) estimates per-execution device time plus
    a per-call dispatch overhead (~400us on this axon setup; calibrate with
    a trivial kernel and subtract).
    """
    import jax
    import numpy as _np
    import concourse.mybir as mybir
    from concourse import bass2jax
    from jax.experimental.shard_map import shard_map
    from jax.sharding import Mesh, PartitionSpec, NamedSharding

    if nc is None:
        if "nc" not in _CACHE:
            _CACHE["nc"] = _build_nc()
        nc = _CACHE["nc"]

    bass2jax.install_neuronx_cc_hook()
    n_cores = 8

    partition_name = nc.partition_id_tensor.name if nc.partition_id_tensor else None
    in_names, out_names, out_avals, zero_outs = [], [], [], []
    for alloc in nc.m.functions[0].allocations:
        if not isinstance(alloc, mybir.MemoryLocationSet):
            continue
        name = alloc.memorylocations[0].name
        if alloc.kind == "ExternalInput":
            if name != partition_name:
                in_names.append(name)
        elif alloc.kind == "ExternalOutput":
            out_names.append(name)
            shape = tuple(alloc.tensor_shape)
            dtype = mybir.dt.np(alloc.dtype)
            out_avals.append(jax.core.ShapedArray(shape, dtype))
            zero_outs.append(_np.zeros(shape, dtype))
    n_params = len(in_names)
    all_names = in_names + out_names
    if partition_name is not None:
        all_names = all_names + [partition_name]

    def _body(*args):
        operands = list(args)
        if partition_name is not None:
            operands.append(bass2jax.partition_id_tensor())
        outs = bass2jax._bass_exec_p.bind(
            *operands,
            out_avals=tuple(out_avals),
            in_names=tuple(all_names),
            out_names=tuple(out_names),
            lowering_input_output_aliases=(),
            sim_require_finite=True,
            sim_require_nnan=True,
            nc=nc,
        )
        return tuple(outs)

    devices = jax.devices()[:n_cores]
    mesh = Mesh(_np.asarray(devices), ("core",))
    spec = PartitionSpec("core")
    sharded = jax.jit(
        shard_map(
            _body,
            mesh=mesh,
            in_specs=(spec,) * (n_params + len(out_names)),
            out_specs=(spec,) * len(out_names),
            check_rep=False,
        ),
        keep_unused=True,
    )
    sh = NamedSharding(mesh, spec)
    dev_args = [
        jax.device_put(
            _np.concatenate([_np.asarray(in_maps[c][nm]) for c in range(n_cores)], 0),
            sh,
        )
        for nm in in_names
    ] + [
        jax.device_put(
            _np.zeros((n_cores * z.shape[0], *z.shape[1:]), z.dtype), sh
        )
        for z in zero_outs
    ]

    import time as _time

    def timed(n):
        out = None
        t0 = _time.perf_counter()
        for _ in range(n):
            out = sharded(*dev_args)
        jax.block_until_ready(out)
        return _time.perf_counter() - t0

    return timed
